# revision 24
# baseline (speedup 1.0000x reference)
# Distributed Bass kernel for nn_DecoderBlock (AdaLN decoder block) on 8 TRN2 cores.
#
# Sharding: core i -> (batch b = i//4, sequence quarter r = i%4, 512 tokens).
# Weights replicated (bf16). The only collective is a 4-rank AllGather of the
# local K^T / V slices per batch group (chunked 4x so attention starts early).
#
# Layout convention: every on-chip activation is stored transposed,
# [features(partitions), tokens(free)], so each linear y = h @ W uses the
# weight (in,out) directly as matmul lhsT and needs no on-chip transposes.
# Host pre-transposes/shards x and cond, folds 1/sqrt(d) into the q columns
# of qkv_w and the AdaLN "+1" into the gamma half of p1_b/p2_b.
#
# Perf notes vs the original baseline (565us -> target ~420us):
#  - tiny warm-up AllGather at t=0 absorbs the collective cold-start that
#    made the first real AllGather run ~3x slower than the rest
#  - input DMAs reordered (cond, x, p1w first) and the adaln1 -> qkv chunk-0
#    chain tightened so the first AllGather triggers at ~30us, not ~55us
#  - adaln normalize applies in [128, 2*T] j-pairs (half the DVE dispatches)
#  - softmax denominators: reciprocal_approx_fast on [2,T] once per head
#    pair (was: 2x vector.reciprocal at 3.3us each), and one sel-matrix
#    matmul broadcasts both heads' reciprocals in a single PE op
#  - score PSUM pool deepened to 3 slots so the scores->exp->AV pipeline
#    doesn't stall the PE
#  - x loaded once; the attention residual adds in place into the same tile
#  - wo/w1 weight DMAs issued before the attention loop so they land during it

import os

os.environ.setdefault("MYCRO_LOCAL_CACHE", "1")

import numpy as np
import ml_dtypes

import concourse.bass as bass
import concourse.mybir as mybir
import concourse.tile as tile
from concourse import bacc
from concourse.bass_utils import run_bass_kernel_spmd

F32 = mybir.dt.float32
BF16 = mybir.dt.bfloat16
FP8 = mybir.dt.float8e4
AF = mybir.ActivationFunctionType
ALU = mybir.AluOpType

D = 1024        # d_model
DC = 512        # d_cond
H = 16          # heads
DH = 64         # head dim
FF = 4096       # ffn dim
T = 512         # tokens per core
S = 2048        # sequence length per batch
B = 2
NCORES = 8
GROUP = 4       # cores per batch group
EPS = 1e-5

_CACHE = {}


def _build():
    nc = bacc.Bacc(
        "TRN2",
        target_bir_lowering=False,
        debug=False,
        enable_asserts=False,
        num_devices=NCORES,
    )

    # ---- DRAM I/O ----
    xT = nc.dram_tensor("xT", [D, T], F32, kind="ExternalInput").ap()
    condT = nc.dram_tensor("condT", [DC, T], BF16, kind="ExternalInput").ap()
    p1w = nc.dram_tensor("p1w", [DC, 2 * D], BF16, kind="ExternalInput").ap()
    p1b = nc.dram_tensor("p1b", [128, 16], F32, kind="ExternalInput").ap()
    qkvw = nc.dram_tensor("qkvw", [D, 3 * D], BF16, kind="ExternalInput").ap()
    wo = nc.dram_tensor("wo", [D, D], BF16, kind="ExternalInput").ap()
    p2w = nc.dram_tensor("p2w", [DC, 2 * D], BF16, kind="ExternalInput").ap()
    p2b = nc.dram_tensor("p2b", [128, 16], F32, kind="ExternalInput").ap()
    w1 = nc.dram_tensor("w1", [D, FF], BF16, kind="ExternalInput").ap()
    b1 = nc.dram_tensor("b1", [128, 32], F32, kind="ExternalInput").ap()
    w2 = nc.dram_tensor("w2", [FF, D], BF16, kind="ExternalInput").ap()
    b2 = nc.dram_tensor("b2", [128, 8], F32, kind="ExternalInput").ap()
    out_d = nc.dram_tensor("out", [D, T], F32, kind="ExternalOutput").ap()

    with tile.TileContext(nc) as tc:
        _emit(nc, tc, xT, condT, p1w, p1b, qkvw, wo, p2w, p2b, w1, b1, w2, b2, out_d)

    nc.compile()
    return nc


def _emit(nc, tc, xT, condT, p1w, p1b, qkvw, wo, p2w, p2b, w1, b1, w2, b2, out_d):
    # Pool lifetimes follow a two-sided stack discipline (LIFO per side):
    # left = phase-nested pools, right = phase-crossing carries.
    def pool(name, bufs=1, space="SBUF", side=None):
        return tc.alloc_tile_pool(name=name, bufs=bufs, space=space, side=side)

    # ---------------- persistent pools ----------------
    const = pool("const")
    work = pool("work", bufs=4)            # [128,T] temporaries
    small = pool("small", bufs=4)          # [1,T] stats
    dram = pool("dram", bufs=1, space="DRAM")

    # right-side carry: x lives to the end; the attention residual adds into
    # it in place, so it doubles as x1.
    x_pool = pool("x_pool", side="right")
    xT_sb = x_pool.tile([128, 8 * T], F32, name="xT_sb")

    # ---------------- warm-up collective ----------------
    # The first AllGather of a NEFF runs far below link rate (ring/descriptor
    # cold start). Fire a tiny one immediately; it has no input dependencies
    # (the data is junk) and overlaps the input DMAs.
    wu_in = dram.tile([128, 16], BF16, name="wu_in")
    wu_out = dram.tile([GROUP, 128, 16], BF16, name="wu_out")
    nc.gpsimd.collective_compute(
        "AllGather",
        ALU.bypass,
        replica_groups=[[0, 1, 2, 3], [4, 5, 6, 7]],
        ins=[wu_in[:]],
        outs=[wu_out[:]],
    )

    # ---------------- constants ----------------
    ones_col_bf = const.tile([128, 1], BF16, name="ones_col_bf")
    nc.vector.memset(ones_col_bf[:], 1.0)
    ones_row_f = const.tile([1, 128], F32, name="ones_row_f")
    nc.vector.memset(ones_row_f[:], 1.0)
    eps_t = const.tile([1, 1], F32, name="eps_t")
    nc.vector.memset(eps_t[:], EPS)
    ones_all = const.tile([128, 64], F32, name="ones_all")
    nc.vector.memset(ones_all[:], 1.0)

    p1b_sb = const.tile([128, 16], F32, name="p1b_sb")
    nc.sync.dma_start(p1b_sb[:], p1b)
    p2b_sb = const.tile([128, 16], F32, name="p2b_sb")
    nc.sync.dma_start(p2b_sb[:], p2b)
    b1_sb = const.tile([128, 32], F32, name="b1_sb")
    nc.sync.dma_start(b1_sb[:], b1)
    b2_sb = const.tile([128, 8], F32, name="b2_sb")
    nc.sync.dma_start(b2_sb[:], b2)

    # ---------------- input loads, priority order ----------------
    # cond gates silu -> gb1; x gates the adaln1 stats; p1w gates gb1;
    # qkvw is needed ~20us in (kv chunk matmuls); p2w only at ~45us.
    cond_pool = pool("cond_pool")
    cond_sb = cond_pool.tile([128, 4 * T], BF16, name="cond_sb")
    for a in range(4):
        nc.sync.dma_start(cond_sb[:, T * a:T * (a + 1)], condT[128 * a:128 * (a + 1), :])
    sc_sb = cond_pool.tile([128, 4 * T], BF16, name="sc_sb")

    for a in range(8):
        nc.sync.dma_start(xT_sb[:, T * a:T * (a + 1)], xT[128 * a:128 * (a + 1), :])

    proj_pool = pool("proj_pool")
    p1w_sb = proj_pool.tile([128, 4 * 2048], BF16, name="p1w_sb")
    for a in range(4):
        nc.sync.dma_start(p1w_sb[:, 2048 * a:2048 * (a + 1)], p1w[128 * a:128 * (a + 1), :])

    qkvw_pool = pool("qkvw_pool")
    qkvw_sb = qkvw_pool.tile([128, 8 * 3072], BF16, name="qkvw_sb")
    for a in range(8):
        nc.sync.dma_start(qkvw_sb[:, 3072 * a:3072 * (a + 1)], qkvw[128 * a:128 * (a + 1), :])

    p2w_sb = proj_pool.tile([128, 4 * 2048], BF16, name="p2w_sb")
    for a in range(4):
        nc.sync.dma_start(p2w_sb[:, 2048 * a:2048 * (a + 1)], p2w[128 * a:128 * (a + 1), :])

    # PSUM pools for the pre-phase: evictions (2) + stats (2) + broadcast (4)
    st_ps = pool("st_ps", bufs=2, space="PSUM")     # [1,T] stats accumulators
    bc_ps = pool("bc_ps", bufs=1, space="PSUM")     # [128, 4T] mu/rs broadcast
    mm_ps = pool("mm_ps", bufs=2, space="PSUM")     # matmul eviction banks

    # ---------------- helpers ----------------
    def adaln(src_f32, gb_sb, h_sb, tmp_pool, stp, bcp, prefix):
        """src_f32: [128, 8*T] f32 ([D, T] transposed); writes h_sb bf16.

        LayerNorm stats via PE ones-matmuls, then normalize+affine applied
        in [128, 2*T] j-pairs (gamma tiles 0..7 and beta tiles 8..15 of
        gb_sb are each contiguous, so pairs slice cleanly).
        """
        src_bf = tmp_pool.tile([128, 8 * T], BF16, name=f"{prefix}src_bf", tag="src_bf")
        sq = tmp_pool.tile([128, 8 * T], BF16, name=f"{prefix}sq", tag="sq")
        for hv in range(2):
            hs = slice(4 * T * hv, 4 * T * (hv + 1))
            nc.vector.tensor_copy(src_bf[:, hs], src_f32[:, hs])
            nc.vector.tensor_mul(sq[:, hs], src_bf[:, hs], src_bf[:, hs])

        sums = stp.tile([1, T], F32, name="sums", tag="st")
        for j in range(8):
            nc.tensor.matmul(sums[:], ones_col_bf[:], src_bf[:, j * T:(j + 1) * T],
                             start=(j == 0), stop=(j == 7))
        sumsq = stp.tile([1, T], F32, name="sumsq", tag="st")
        for j in range(8):
            nc.tensor.matmul(sumsq[:], ones_col_bf[:], sq[:, j * T:(j + 1) * T],
                             start=(j == 0), stop=(j == 7))

        mu = small.tile([1, T], F32, name="mu", tag="sm")
        nc.vector.tensor_scalar_mul(mu[:], sums[:], 1.0 / D)
        musq = small.tile([1, T], F32, name="musq", tag="sm")
        nc.vector.tensor_mul(musq[:], mu[:], mu[:])
        var = small.tile([1, T], F32, name="var", tag="sm")
        nc.vector.scalar_tensor_tensor(var[:], sumsq[:], 1.0 / D, musq[:],
                                       op0=ALU.mult, op1=ALU.subtract)
        lnv = small.tile([1, T], F32, name="lnv", tag="sm")
        nc.scalar.activation(lnv[:], var[:], AF.Ln, bias=eps_t[:], scale=1.0)
        rs = small.tile([1, T], F32, name="rs", tag="sm")
        nc.scalar.activation(rs[:], lnv[:], AF.Exp, scale=-0.5)

        # broadcast mu and rs to 128 partitions, each repeated twice along
        # the free dim so the j-pair ops below can consume them directly
        mrs_b = bcp.tile([128, 4 * T], F32, name="mrs_b", tag="bc")
        for r in range(2):
            nc.tensor.matmul(mrs_b[:, r * T:(r + 1) * T], ones_row_f[:], mu[:],
                             start=True, stop=True)
            nc.tensor.matmul(mrs_b[:, (2 + r) * T:(3 + r) * T], ones_row_f[:], rs[:],
                             start=True, stop=True)
        mrs_bs = tmp_pool.tile([128, 4 * T], BF16, name=f"{prefix}mrs_bs", tag="mrs")
        nc.vector.tensor_copy(mrs_bs[:], mrs_b[:])
        mu2 = mrs_bs[:, 0:2 * T]
        rs2 = mrs_bs[:, 2 * T:4 * T]

        for jp in range(0, 8, 2):
            sl = slice(jp * T, (jp + 2) * T)
            bsl = slice((8 + jp) * T, (10 + jp) * T)
            t1 = work.tile([128, 2 * T], BF16, name="t1", tag="wk2")
            nc.vector.tensor_sub(t1[:], src_bf[:, sl], mu2)
            t2 = work.tile([128, 2 * T], BF16, name="t2", tag="wk2")
            nc.vector.tensor_mul(t2[:], t1[:], rs2)
            t3 = work.tile([128, 2 * T], BF16, name="t3", tag="wk2")
            nc.vector.tensor_mul(t3[:], t2[:], gb_sb[:, sl])           # *(1+gamma)
            nc.vector.tensor_add(h_sb[:, sl], t3[:], gb_sb[:, bsl])

    def proj_gb(w_sb, b_sb, gb_sb):
        """gb^T = (silu(cond) @ W + b)^T : 16 M-tiles of [128, T]."""
        for m in range(16):
            ps = mm_ps.tile([128, T], F32, name="gbps", tag="mm")
            for k in range(4):
                nc.tensor.matmul(ps[:], w_sb[:, 2048 * k + 128 * m: 2048 * k + 128 * (m + 1)],
                                 sc_sb[:, k * T:(k + 1) * T],
                                 start=(k == 0), stop=(k == 3))
            nc.vector.tensor_scalar_add(gb_sb[:, m * T:(m + 1) * T], ps[:],
                                        b_sb[:, m:m + 1])

    # ---------------- AdaLN 1 ----------------
    for a in range(4):
        sl = slice(T * a, T * (a + 1))
        nc.scalar.activation(sc_sb[:, sl], cond_sb[:, sl], AF.Silu)

    h1_pool = pool("h1_pool")
    h1_sb = h1_pool.tile([128, 8 * T], BF16, name="h1_sb")

    gb1_pool = pool("gb1_pool")
    gb1_sb = gb1_pool.tile([128, 16 * T], BF16, name="gb1_sb")
    proj_gb(p1w_sb, p1b_sb, gb1_sb)

    aln1_tmp = pool("aln1_tmp")
    adaln(xT_sb, gb1_sb, h1_sb, aln1_tmp, st_ps, bc_ps, "a1")
    aln1_tmp.release()
    gb1_pool.release()

    # ---------------- qkv + chunked collective ----------------
    # 4 AllGather chunks, one per 4-head group: chunk c carries k^T feature
    # rows [256c:256c+256] and v columns [256c:256c+256], so attention on
    # head-pairs 2c,2c+1 can start while later chunks are still in flight.
    kv_pool = pool("kv_pool")
    kT_loc = kv_pool.tile([128, 8 * T], BF16, name="kT_loc")
    v_loc = kv_pool.tile([128, 4 * D], BF16, name="v_loc")

    NCH = 4
    kv_ins = [dram.tile([512, T], BF16, name=f"kv_in{c}") for c in range(NCH)]
    kv_outs = [dram.tile([GROUP, 512, T], BF16, name=f"kv_out{c}") for c in range(NCH)]

    for c in range(NCH):
        # k^T feature M-tiles for heads 4c..4c+3
        for ml in range(2):
            m = 8 + 2 * c + ml
            ps = mm_ps.tile([128, T], F32, name="kps", tag="mm")
            for k in range(8):
                nc.tensor.matmul(ps[:], qkvw_sb[:, 3072 * k + 128 * m: 3072 * k + 128 * (m + 1)],
                                 h1_sb[:, k * T:(k + 1) * T],
                                 start=(k == 0), stop=(k == 7))
            nc.vector.tensor_copy(kT_loc[:, (m - 8) * T:(m - 7) * T], ps[:])
        # v quarter c ([tokens, 256 features]), token M-tiles
        for mt in range(4):
            ps = mm_ps.tile([128, 256], F32, name="vps", tag="mm")
            for k in range(8):
                nc.tensor.matmul(
                    ps[:],
                    h1_sb[:, k * T + 128 * mt: k * T + 128 * (mt + 1)],
                    qkvw_sb[:, 3072 * k + 2048 + 256 * c: 3072 * k + 2048 + 256 * (c + 1)],
                    start=(k == 0), stop=(k == 7))
            nc.vector.tensor_copy(v_loc[:, 1024 * mt + 256 * c: 1024 * mt + 256 * (c + 1)], ps[:])
        # bounce writes + collective for this chunk
        for ml in range(2):
            nc.sync.dma_start(kv_ins[c][128 * ml:128 * (ml + 1), :],
                              kT_loc[:, (2 * c + ml) * T:(2 * c + ml + 1) * T])
        vdst = kv_ins[c][256:512, :].rearrange("r (two f) -> (r two) f", two=2)
        nc.sync.dma_start(vdst.rearrange("(m p) f -> p m f", m=4),
                          v_loc.rearrange("p (m f) -> p m f", m=4)[:, :, 256 * c:256 * (c + 1)])
        nc.gpsimd.collective_compute(
            "AllGather",
            ALU.bypass,
            replica_groups=[[0, 1, 2, 3], [4, 5, 6, 7]],
            ins=[kv_ins[c][:]],
            outs=[kv_outs[c][:]],
        )
    kv_pool.release()

    # right-side carries for the attention phase
    gb2_pool = pool("gb2_pool", side="right")
    gb2_sb = gb2_pool.tile([128, 16 * T], BF16, name="gb2_sb")
    oT_pool = pool("oT_pool", side="right")
    oT_sb = oT_pool.tile([128, 8 * T], BF16, name="oT_sb")
    q_pool = pool("q_pool", side="right")
    qT_sb = q_pool.tile([128, 8 * T], BF16, name="qT_sb")

    # q^T (feature M-tiles 0..7), overlaps with collective
    for m in range(8):
        ps = mm_ps.tile([128, T], F32, name="qps", tag="mm")
        for k in range(8):
            nc.tensor.matmul(ps[:], qkvw_sb[:, 3072 * k + 128 * m: 3072 * k + 128 * (m + 1)],
                             h1_sb[:, k * T:(k + 1) * T],
                             start=(k == 0), stop=(k == 7))
        nc.vector.tensor_copy(qT_sb[:, m * T:(m + 1) * T], ps[:])

    # gb2 projection, overlaps with collective
    proj_gb(p2w_sb, p2b_sb, gb2_sb)

    h1_pool.release()
    mm_ps.release()
    bc_ps.release()
    st_ps.release()
    qkvw_pool.release()
    proj_pool.release()
    cond_pool.release()

    # ---------------- attention ----------------
    att_pool = pool("att_pool")
    kT_full = att_pool.tile([128, 8 * S], BF16, name="kT_full")
    VW = DH + 1  # 65: per-head V columns + ones column (softmax denominator)
    # fp8 V + softmax weights: halves attention-phase SBUF traffic; the
    # ones column stays exactly 1.0 so the denominator matches the numerator
    vext = att_pool.tile([128, 16 * H * VW], FP8, name="vext")
    vext_v = vext.rearrange("p (c h m) -> p c h m", c=16, m=VW)
    nc.vector.memset(vext_v[:, :, :, DH:DH + 1], 1.0)

    def readback_chunk(c):
        for fl in range(2):
            f = 2 * c + fl
            for r in range(GROUP):
                nc.sync.dma_start(kT_full[:, 2048 * f + 512 * r: 2048 * f + 512 * (r + 1)],
                                  kv_outs[c][r, 128 * fl:128 * (fl + 1), :])
        for r in range(GROUP):
            vch = kv_outs[c][r, 256:512, :].rearrange("q (two f) -> (q two) f", two=2)
            for lc in range(4):
                c2 = 4 * r + lc
                src = vch[128 * lc:128 * (lc + 1), :].rearrange("t (h d) -> t h d", d=DH)
                # SWDGE queue: keeps vext readbacks off the sync DMA queues so
                # they don't serialize behind later chunks' waits
                nc.gpsimd.dma_start(vext_v[:, c2, 4 * c:4 * (c + 1), 0:DH], src)

    # emit all readbacks now, BEFORE any later weight loads: each chunk's
    # transfers wait on its AllGather semaphore, and weight bytes queued
    # ahead of them would delay the attention-critical data
    for c in range(NCH):
        readback_chunk(c)

    p_pool = pool("p_pool", bufs=4)
    norm_pool = pool("norm_pool", bufs=2)
    sc_ps = pool("sc_ps", bufs=3, space="PSUM")     # [128,1024] = 2 banks each
    o_ps_pool = pool("o_ps", bufs=2, space="PSUM")

    norm_pending = []

    def after_av(pv_hp, o_tiles):
        # Part A (DVE only): evict raw o^T + denominator (freeing o psum
        # quickly) and compute a fast-approx reciprocal; the PE-side broadcast
        # runs a pair later via flush_norm so the reciprocal chain never
        # stalls the in-order PE queue. Denominators live at partition 64
        # (engine partition starts must be 32-aligned).
        for hh in range(2):
            nc.vector.tensor_copy(oT_sb[64 * hh:64 * (hh + 1), pv_hp * T:(pv_hp + 1) * T],
                                  o_tiles[hh][0:DH, :])
            den = norm_pool.tile([128, T], F32, name="den", tag="den")
            nc.vector.tensor_copy(den[64:65, :], o_tiles[hh][DH:DH + 1, :])
            rec = norm_pool.tile([128, T], F32, name="rec", tag="rec", bufs=4)
            # full-tile op: the custom-DVE lowering needs a partition-0 start;
            # only row 64 is ever read downstream (other rows are junk/junk)
            nc.vector.reciprocal_approx_fast(out=rec[:], in_=den[:])
            norm_pending.append((pv_hp, hh, rec))

    def flush_norm():
        for (php, phh, rec) in norm_pending:
            rbt = sc_ps.tile([128, 2 * T], F32, name="rbt", tag="s")
            nc.tensor.matmul(rbt[0:64, 0:T], ones_all[64:65, :], rec[64:65, :],
                             start=True, stop=True)
            rb_sb = norm_pool.tile([128, T], BF16, name="rb_sb", tag="rbs")
            nc.vector.tensor_copy(rb_sb[64 * phh:64 * (phh + 1), :], rbt[0:64, 0:T])
            osl = oT_sb[64 * phh:64 * (phh + 1), php * T:(php + 1) * T]
            nc.vector.tensor_mul(osl, osl, rb_sb[64 * phh:64 * (phh + 1), :])
        norm_pending.clear()

    prev = None
    for hp in range(8):
        p_tiles = [p_pool.tile([128, 16 * T], FP8, name=f"pt{hh}", tag="p") for hh in range(2)]
        q_h = [qT_sb[64 * hh:64 * (hh + 1), hp * T:(hp + 1) * T] for hh in range(2)]
        o_tiles = None
        if prev is not None:
            o_tiles = [o_ps_pool.tile([128, T], F32, name="o_ps", tag="o") for _ in range(2)]
        # 8 groups: scores for chunks (2m2, 2m2+1) of both heads, interleaved
        # with 4 AV matmuls of the previous pair so PE work overlaps ACT exp.
        for m2 in range(8):
            scts = [sc_ps.tile([128, 1024], F32, name="sct", tag="s") for _ in range(2)]
            for half in range(2):
                m = 2 * m2 + half
                for hh in range(2):
                    rows = slice(64 * hh, 64 * (hh + 1))
                    nc.tensor.matmul(
                        scts[hh][:, 512 * half:512 * (half + 1)],
                        kT_full[rows, 2048 * hp + 128 * m: 2048 * hp + 128 * (m + 1)],
                        q_h[hh],
                        start=True, stop=True)
            if prev is not None:
                pv_tiles, pv_hp = prev
                for hh in range(2):
                    h = 2 * pv_hp + hh
                    for half in range(2):
                        cc = 2 * m2 + half
                        nc.tensor.matmul(
                            o_tiles[hh][0:VW, :],
                            vext[:, VW * (16 * cc + h): VW * (16 * cc + h) + VW],
                            pv_tiles[hh][:, cc * T:(cc + 1) * T],
                            start=(cc == 0), stop=(cc == 15))
            for hh in range(2):
                nc.scalar.activation(p_tiles[hh][:, 2 * m2 * T:(2 * m2 + 2) * T],
                                     scts[hh][:], AF.Exp)
        flush_norm()
        if prev is not None:
            after_av(prev[1], o_tiles)
        prev = (p_tiles, hp)

    # tail: AV + normalize for the last pair
    pv_tiles, pv_hp = prev
    o_tiles = [o_ps_pool.tile([128, T], F32, name="o_ps", tag="o") for _ in range(2)]
    for cc in range(16):
        for hh in range(2):
            h = 2 * pv_hp + hh
            nc.tensor.matmul(
                o_tiles[hh][0:VW, :],
                vext[:, VW * (16 * cc + h): VW * (16 * cc + h) + VW],
                pv_tiles[hh][:, cc * T:(cc + 1) * T],
                start=(cc == 0), stop=(cc == 15))
    flush_norm()
    after_av(pv_hp, o_tiles)
    flush_norm()

    o_ps_pool.release()
    sc_ps.release()
    norm_pool.release()
    p_pool.release()
    att_pool.release()

    # ---------------- attn_out + residual (in place into xT_sb) ----------
    mm_ps2 = pool("mm_ps2", bufs=2, space="PSUM")
    st_ps2 = pool("st_ps2", bufs=2, space="PSUM")
    bc_ps2 = pool("bc_ps2", bufs=1, space="PSUM")

    wo_pool = pool("wo_pool")
    wo_sb = wo_pool.tile([128, 8 * D], BF16, name="wo_sb")
    for a in range(8):
        nc.sync.dma_start(wo_sb[:, 1024 * a:1024 * (a + 1)], wo[128 * a:128 * (a + 1), :])

    for m in range(8):
        ps = mm_ps2.tile([128, T], F32, name="aops", tag="mm")
        for k in range(8):
            nc.tensor.matmul(ps[:], wo_sb[:, 1024 * k + 128 * m: 1024 * k + 128 * (m + 1)],
                             oT_sb[:, k * T:(k + 1) * T],
                             start=(k == 0), stop=(k == 7))
        nc.vector.tensor_add(xT_sb[:, m * T:(m + 1) * T], ps[:], xT_sb[:, m * T:(m + 1) * T])
    wo_pool.release()

    q_pool.release()
    oT_pool.release()

    # ---------------- AdaLN 2 ----------------
    g_pool = pool("g_pool")
    g_sb = g_pool.tile([128, 32 * T], BF16, name="g_sb")

    h2_pool = pool("h2_pool")
    h2_sb = h2_pool.tile([128, 8 * T], BF16, name="h2_sb")

    w1_pool = pool("w1_pool")
    w1_sb = w1_pool.tile([128, 8 * FF], BF16, name="w1_sb")
    for a in range(8):
        nc.sync.dma_start(w1_sb[:, 4096 * a:4096 * (a + 1)], w1[128 * a:128 * (a + 1), :])

    aln2_tmp = pool("aln2_tmp")
    adaln(xT_sb, gb2_sb, h2_sb, aln2_tmp, st_ps2, bc_ps2, "a2")
    aln2_tmp.release()
    gb2_pool.release()
    bc_ps2.release()
    st_ps2.release()

    # ---------------- FFN ----------------
    for m in range(32):
        ps = mm_ps2.tile([128, T], F32, name="f1ps", tag="mm")
        for k in range(8):
            nc.tensor.matmul(ps[:], w1_sb[:, 4096 * k + 128 * m: 4096 * k + 128 * (m + 1)],
                             h2_sb[:, k * T:(k + 1) * T],
                             start=(k == 0), stop=(k == 7))
        nc.scalar.activation(g_sb[:, m * T:(m + 1) * T], ps[:], AF.Gelu,
                             bias=b1_sb[:, m:m + 1], scale=1.0)
    w1_pool.release()
    h2_pool.release()
    mm_ps2.release()

    # ffn2: k-outer, stream w2 k-tiles; two m-halves so the first half's
    # evictions overlap the second half's matmuls
    w2_pool = pool("w2_pool", bufs=4)
    ff2_ps = pool("ff2_ps", bufs=1, space="PSUM")
    out_pool0 = pool("out_pool0")
    out_sb = out_pool0.tile([128, 8 * T], F32, name="out_sb")
    for half in range(2):
        o2 = [ff2_ps.tile([128, T], F32, name=f"ff2_{m}", tag=f"ff2_{m}") for m in range(4)]
        for k in range(32):
            w2t = w2_pool.tile([128, 512], BF16, name="w2t", tag="w2t")
            nc.sync.dma_start(w2t[:], w2[128 * k: 128 * (k + 1), 512 * half:512 * (half + 1)])
            for m in range(4):
                nc.tensor.matmul(o2[m][:], w2t[:, 128 * m: 128 * (m + 1)],
                                 g_sb[:, k * T:(k + 1) * T],
                                 start=(k == 0), stop=(k == 31))
        for m in range(4):
            gm = 4 * half + m
            nc.vector.scalar_tensor_tensor(out_sb[:, gm * T:(gm + 1) * T], o2[m][:],
                                           b2_sb[:, gm:gm + 1], xT_sb[:, gm * T:(gm + 1) * T],
                                           op0=ALU.add, op1=ALU.add)
        for a in range(4 * half, 4 * half + 4):
            nc.sync.dma_start(out_d[128 * a:128 * (a + 1), :], out_sb[:, T * a:T * (a + 1)])

    out_pool0.release()
    ff2_ps.release()
    w2_pool.release()
    g_pool.release()
    x_pool.release()
    small.release()
    work.release()
    const.release()
    dram.release()


def _bf16(a):
    return np.ascontiguousarray(a).astype(ml_dtypes.bfloat16)


def _prep_maps(x, cond, p1_w, p1_b, qkv_w, attn_out_w, p2_w, p2_b,
               ffn_w1, ffn_b1, ffn_w2, ffn_b2):
    x = np.asarray(x, np.float32)
    cond = np.asarray(cond, np.float32)
    qkv_mod = np.asarray(qkv_w, np.float32).copy()
    qkv_mod[:, :D] *= DH ** -0.5                      # fold 1/sqrt(d) into q
    p1b_mod = np.asarray(p1_b, np.float32).copy()
    p1b_mod[:D] += 1.0                                # fold AdaLN "+1" into gamma bias
    p2b_mod = np.asarray(p2_b, np.float32).copy()
    p2b_mod[:D] += 1.0

    shared = {
        "p1w": _bf16(p1_w),
        "p1b": np.ascontiguousarray(p1b_mod.reshape(16, 128).T, np.float32),
        "qkvw": _bf16(qkv_mod),
        "wo": _bf16(attn_out_w),
        "p2w": _bf16(p2_w),
        "p2b": np.ascontiguousarray(p2b_mod.reshape(16, 128).T, np.float32),
        "w1": _bf16(ffn_w1),
        "b1": np.ascontiguousarray(np.asarray(ffn_b1, np.float32).reshape(32, 128).T,
                                   np.float32),
        "w2": _bf16(ffn_w2),
        "b2": np.ascontiguousarray(np.asarray(ffn_b2, np.float32).reshape(8, 128).T,
                                   np.float32),
    }
    in_maps = []
    for core in range(NCORES):
        b, r = core // GROUP, core % GROUP
        sl = slice(T * r, T * (r + 1))
        m = dict(shared)
        m["xT"] = np.ascontiguousarray(x[b, sl, :].T, np.float32)
        m["condT"] = _bf16(cond[b, sl, :].T)
        in_maps.append(m)
    return in_maps


def _get_nc():
    if "nc" not in _CACHE:
        _CACHE["nc"] = _build()
    return _CACHE["nc"]


def _install_ntff_hook():
    """This image's antenv lacks axon_hooks; recreate it (see trn_boot.py)."""
    import sys, types, ctypes, contextlib

    if "antenv.axon_hooks" in sys.modules:
        return
    mod = types.ModuleType("antenv.axon_hooks")
    state = {"hook": None}
    mod.set_axon_ntff_profile_hook = lambda h: state.__setitem__("hook", h)
    mod.get_axon_ntff_profile_hook = lambda: state["hook"]
    sys.modules["antenv.axon_hooks"] = mod
    try:
        import antenv
        antenv.axon_hooks = mod
    except ImportError:
        pass

    so_path = "/opt/axon/libaxon_pjrt.so"
    if not os.path.exists(so_path):
        return
    lib = ctypes.CDLL(so_path)
    if not hasattr(lib, "axon_start_nrt_profile"):
        return
    lib.axon_start_nrt_profile.argtypes = [ctypes.POINTER(ctypes.c_int64), ctypes.c_size_t]
    lib.axon_start_nrt_profile.restype = ctypes.c_int64
    lib.axon_stop_nrt_profile.argtypes = [ctypes.c_char_p]
    lib.axon_stop_nrt_profile.restype = ctypes.c_int64

    @contextlib.contextmanager
    def _hook(output_dir, device_ids):
        import jax
        jax.devices()
        if device_ids:
            ids = (ctypes.c_int64 * len(device_ids))(*device_ids)
            rc = lib.axon_start_nrt_profile(ids, len(device_ids))
        else:
            rc = lib.axon_start_nrt_profile(None, 0)
        if rc != 0:
            raise RuntimeError(f"axon_start_nrt_profile rc={rc}")
        try:
            yield
        finally:
            n = lib.axon_stop_nrt_profile(str(output_dir).encode())
            print(f"ntff profile: {n} file(s) -> {output_dir}")

    mod.set_axon_ntff_profile_hook(_hook)


def run(in_maps, trace=False, **kw):
    if trace:
        _install_ntff_hook()
    nc = _get_nc()
    return run_bass_kernel_spmd(nc, in_maps, core_ids=list(range(NCORES)),
                                trace=trace, **kw)


def kernel(**inputs):
    in_maps = _prep_maps(**inputs)
    res = run(in_maps).results
    out = np.empty((B, S, D), np.float32)
    for core in range(NCORES):
        b, r = core // GROUP, core % GROUP
        out[b, T * r: T * (r + 1), :] = res[core]["out"].T
    return out


# revision 25
# speedup vs baseline: 1.0915x; 1.0915x over previous
# Distributed Bass kernel for nn_DecoderBlock (AdaLN decoder block) on 8 TRN2 cores.
#
# Sharding: core i -> (batch b = i//4, sequence quarter r = i%4, 512 tokens).
# Weights replicated (bf16). The only collective is a 4-rank AllGather of the
# local K^T / V slices per batch group (chunked 4x so attention starts early).
#
# Layout convention: every on-chip activation is stored transposed,
# [features(partitions), tokens(free)], so each linear y = h @ W uses the
# weight (in,out) directly as matmul lhsT and needs no on-chip transposes.
# Host pre-transposes/shards x and cond, folds 1/sqrt(d) into the q columns
# of qkv_w and the AdaLN "+1" into the gamma half of p1_b/p2_b.
#
# Perf notes vs the original baseline (565us -> target ~420us):
#  - tiny warm-up AllGather at t=0 absorbs the collective cold-start that
#    made the first real AllGather run ~3x slower than the rest
#  - input DMAs reordered (cond, x, p1w first) and the adaln1 -> qkv chunk-0
#    chain tightened so the first AllGather triggers at ~30us, not ~55us
#  - adaln normalize applies in [128, 2*T] j-pairs (half the DVE dispatches)
#  - softmax denominators: reciprocal_approx_fast on [2,T] once per head
#    pair (was: 2x vector.reciprocal at 3.3us each), and one sel-matrix
#    matmul broadcasts both heads' reciprocals in a single PE op
#  - score PSUM pool deepened to 3 slots so the scores->exp->AV pipeline
#    doesn't stall the PE
#  - x loaded once; the attention residual adds in place into the same tile
#  - wo/w1 weight DMAs issued before the attention loop so they land during it

import os

os.environ.setdefault("MYCRO_LOCAL_CACHE", "1")

import numpy as np
import ml_dtypes

import concourse.bass as bass
import concourse.mybir as mybir
import concourse.tile as tile
from concourse import bacc
from concourse.bass_utils import run_bass_kernel_spmd

F32 = mybir.dt.float32
BF16 = mybir.dt.bfloat16
FP8 = mybir.dt.float8e4
AF = mybir.ActivationFunctionType
ALU = mybir.AluOpType

D = 1024        # d_model
DC = 512        # d_cond
H = 16          # heads
DH = 64         # head dim
FF = 4096       # ffn dim
T = 512         # tokens per core
S = 2048        # sequence length per batch
B = 2
NCORES = 8
GROUP = 4       # cores per batch group
EPS = 1e-5

_CACHE = {}


def _build():
    nc = bacc.Bacc(
        "TRN2",
        target_bir_lowering=False,
        debug=False,
        enable_asserts=False,
        num_devices=NCORES,
    )

    # ---- DRAM I/O ----
    xT = nc.dram_tensor("xT", [D, T], F32, kind="ExternalInput").ap()
    condT = nc.dram_tensor("condT", [DC, T], BF16, kind="ExternalInput").ap()
    p1w = nc.dram_tensor("p1w", [DC, 2 * D], BF16, kind="ExternalInput").ap()
    p1b = nc.dram_tensor("p1b", [128, 16], F32, kind="ExternalInput").ap()
    qkvw = nc.dram_tensor("qkvw", [D, 3 * D], BF16, kind="ExternalInput").ap()
    wo = nc.dram_tensor("wo", [D, D], BF16, kind="ExternalInput").ap()
    p2w = nc.dram_tensor("p2w", [DC, 2 * D], BF16, kind="ExternalInput").ap()
    p2b = nc.dram_tensor("p2b", [128, 16], F32, kind="ExternalInput").ap()
    w1 = nc.dram_tensor("w1", [D, FF], BF16, kind="ExternalInput").ap()
    b1 = nc.dram_tensor("b1", [128, 32], F32, kind="ExternalInput").ap()
    w2 = nc.dram_tensor("w2", [FF, D], BF16, kind="ExternalInput").ap()
    b2 = nc.dram_tensor("b2", [128, 8], F32, kind="ExternalInput").ap()
    out_d = nc.dram_tensor("out", [D, T], F32, kind="ExternalOutput").ap()

    with tile.TileContext(nc) as tc:
        _emit(nc, tc, xT, condT, p1w, p1b, qkvw, wo, p2w, p2b, w1, b1, w2, b2, out_d)

    nc.compile()
    return nc


def _emit(nc, tc, xT, condT, p1w, p1b, qkvw, wo, p2w, p2b, w1, b1, w2, b2, out_d):
    # Pool lifetimes follow a two-sided stack discipline (LIFO per side):
    # left = phase-nested pools, right = phase-crossing carries.
    def pool(name, bufs=1, space="SBUF", side=None):
        return tc.alloc_tile_pool(name=name, bufs=bufs, space=space, side=side)

    # ---------------- persistent pools ----------------
    const = pool("const")
    work = pool("work", bufs=4)            # [128,T] temporaries
    small = pool("small", bufs=4)          # [1,T] stats
    dram = pool("dram", bufs=1, space="DRAM")

    # right-side carry: x lives to the end; the attention residual adds into
    # it in place, so it doubles as x1.
    x_pool = pool("x_pool", side="right")
    xT_sb = x_pool.tile([128, 8 * T], F32, name="xT_sb")

    # ---------------- warm-up collective ----------------
    # The first AllGather of a NEFF runs far below link rate (ring/descriptor
    # cold start). Fire a tiny one immediately; it has no input dependencies
    # (the data is junk) and overlaps the input DMAs.
    wu_in = dram.tile([128, 16], BF16, name="wu_in")
    wu_out = dram.tile([GROUP, 128, 16], BF16, name="wu_out")
    nc.gpsimd.collective_compute(
        "AllGather",
        ALU.bypass,
        replica_groups=[[0, 1, 2, 3], [4, 5, 6, 7]],
        ins=[wu_in[:]],
        outs=[wu_out[:]],
    )

    # ---------------- constants ----------------
    ones_col_bf = const.tile([128, 1], BF16, name="ones_col_bf")
    nc.vector.memset(ones_col_bf[:], 1.0)
    ones_row_f = const.tile([1, 128], F32, name="ones_row_f")
    nc.vector.memset(ones_row_f[:], 1.0)
    eps_t = const.tile([1, 1], F32, name="eps_t")
    nc.vector.memset(eps_t[:], EPS)
    ones_all = const.tile([128, 64], F32, name="ones_all")
    nc.vector.memset(ones_all[:], 1.0)

    p1b_sb = const.tile([128, 16], F32, name="p1b_sb")
    nc.sync.dma_start(p1b_sb[:], p1b)
    p2b_sb = const.tile([128, 16], F32, name="p2b_sb")
    nc.sync.dma_start(p2b_sb[:], p2b)
    b1_sb = const.tile([128, 32], F32, name="b1_sb")
    nc.sync.dma_start(b1_sb[:], b1)
    b2_sb = const.tile([128, 8], F32, name="b2_sb")
    nc.sync.dma_start(b2_sb[:], b2)

    # ---------------- input loads, priority order ----------------
    # cond gates silu -> gb1; x gates the adaln1 stats; p1w gates gb1;
    # qkvw is needed ~20us in (kv chunk matmuls); p2w only at ~45us.
    cond_pool = pool("cond_pool")
    cond_sb = cond_pool.tile([128, 4 * T], BF16, name="cond_sb")
    for a in range(4):
        nc.sync.dma_start(cond_sb[:, T * a:T * (a + 1)], condT[128 * a:128 * (a + 1), :])
    sc_sb = cond_pool.tile([128, 4 * T], BF16, name="sc_sb")

    for a in range(8):
        nc.sync.dma_start(xT_sb[:, T * a:T * (a + 1)], xT[128 * a:128 * (a + 1), :])

    proj_pool = pool("proj_pool")
    p1w_sb = proj_pool.tile([128, 4 * 2048], BF16, name="p1w_sb")
    for a in range(4):
        nc.sync.dma_start(p1w_sb[:, 2048 * a:2048 * (a + 1)], p1w[128 * a:128 * (a + 1), :])

    qkvw_pool = pool("qkvw_pool")
    qkvw_sb = qkvw_pool.tile([128, 8 * 3072], BF16, name="qkvw_sb")
    for a in range(8):
        nc.sync.dma_start(qkvw_sb[:, 3072 * a:3072 * (a + 1)], qkvw[128 * a:128 * (a + 1), :])

    p2w_sb = proj_pool.tile([128, 4 * 2048], BF16, name="p2w_sb")
    for a in range(4):
        nc.sync.dma_start(p2w_sb[:, 2048 * a:2048 * (a + 1)], p2w[128 * a:128 * (a + 1), :])

    # PSUM pools for the pre-phase: evictions (2) + stats (2) + broadcast (4)
    st_ps = pool("st_ps", bufs=2, space="PSUM")     # [1,T] stats accumulators
    bc_ps = pool("bc_ps", bufs=1, space="PSUM")     # [128, 4T] mu/rs broadcast
    mm_ps = pool("mm_ps", bufs=2, space="PSUM")     # matmul eviction banks

    # ---------------- helpers ----------------
    def adaln(src_f32, gb_sb, h_sb, tmp_pool, stp, bcp, prefix):
        """src_f32: [128, 8*T] f32 ([D, T] transposed); writes h_sb bf16.

        LayerNorm stats via PE ones-matmuls, then normalize+affine applied
        in [128, 2*T] j-pairs (gamma tiles 0..7 and beta tiles 8..15 of
        gb_sb are each contiguous, so pairs slice cleanly).
        """
        src_bf = tmp_pool.tile([128, 8 * T], BF16, name=f"{prefix}src_bf", tag="src_bf")
        sq = tmp_pool.tile([128, 8 * T], BF16, name=f"{prefix}sq", tag="sq")
        for hv in range(2):
            hs = slice(4 * T * hv, 4 * T * (hv + 1))
            nc.vector.tensor_copy(src_bf[:, hs], src_f32[:, hs])
            nc.vector.tensor_mul(sq[:, hs], src_bf[:, hs], src_bf[:, hs])

        sums = stp.tile([1, T], F32, name="sums", tag="st")
        for j in range(8):
            nc.tensor.matmul(sums[:], ones_col_bf[:], src_bf[:, j * T:(j + 1) * T],
                             start=(j == 0), stop=(j == 7))
        sumsq = stp.tile([1, T], F32, name="sumsq", tag="st")
        for j in range(8):
            nc.tensor.matmul(sumsq[:], ones_col_bf[:], sq[:, j * T:(j + 1) * T],
                             start=(j == 0), stop=(j == 7))

        mu = small.tile([1, T], F32, name="mu", tag="sm")
        nc.vector.tensor_scalar_mul(mu[:], sums[:], 1.0 / D)
        musq = small.tile([1, T], F32, name="musq", tag="sm")
        nc.vector.tensor_mul(musq[:], mu[:], mu[:])
        var = small.tile([1, T], F32, name="var", tag="sm")
        nc.vector.scalar_tensor_tensor(var[:], sumsq[:], 1.0 / D, musq[:],
                                       op0=ALU.mult, op1=ALU.subtract)
        lnv = small.tile([1, T], F32, name="lnv", tag="sm")
        nc.scalar.activation(lnv[:], var[:], AF.Ln, bias=eps_t[:], scale=1.0)
        rs = small.tile([1, T], F32, name="rs", tag="sm")
        nc.scalar.activation(rs[:], lnv[:], AF.Exp, scale=-0.5)

        # broadcast mu and rs to 128 partitions, each repeated twice along
        # the free dim so the j-pair ops below can consume them directly
        mrs_b = bcp.tile([128, 4 * T], F32, name="mrs_b", tag="bc")
        for r in range(2):
            nc.tensor.matmul(mrs_b[:, r * T:(r + 1) * T], ones_row_f[:], mu[:],
                             start=True, stop=True)
            nc.tensor.matmul(mrs_b[:, (2 + r) * T:(3 + r) * T], ones_row_f[:], rs[:],
                             start=True, stop=True)
        mrs_bs = tmp_pool.tile([128, 4 * T], BF16, name=f"{prefix}mrs_bs", tag="mrs")
        nc.vector.tensor_copy(mrs_bs[:], mrs_b[:])
        mu2 = mrs_bs[:, 0:2 * T]
        rs2 = mrs_bs[:, 2 * T:4 * T]

        for jp in range(0, 8, 2):
            sl = slice(jp * T, (jp + 2) * T)
            bsl = slice((8 + jp) * T, (10 + jp) * T)
            t1 = work.tile([128, 2 * T], BF16, name="t1", tag="wk2")
            nc.vector.tensor_sub(t1[:], src_bf[:, sl], mu2)
            t2 = work.tile([128, 2 * T], BF16, name="t2", tag="wk2")
            nc.vector.tensor_mul(t2[:], t1[:], rs2)
            t3 = work.tile([128, 2 * T], BF16, name="t3", tag="wk2")
            nc.vector.tensor_mul(t3[:], t2[:], gb_sb[:, sl])           # *(1+gamma)
            nc.vector.tensor_add(h_sb[:, sl], t3[:], gb_sb[:, bsl])

    def proj_gb(w_sb, b_sb, gb_sb):
        """gb^T = (silu(cond) @ W + b)^T : 16 M-tiles of [128, T]."""
        for m in range(16):
            ps = mm_ps.tile([128, T], F32, name="gbps", tag="mm")
            for k in range(4):
                nc.tensor.matmul(ps[:], w_sb[:, 2048 * k + 128 * m: 2048 * k + 128 * (m + 1)],
                                 sc_sb[:, k * T:(k + 1) * T],
                                 start=(k == 0), stop=(k == 3))
            nc.vector.tensor_scalar_add(gb_sb[:, m * T:(m + 1) * T], ps[:],
                                        b_sb[:, m:m + 1])

    # ---------------- AdaLN 1 ----------------
    for a in range(4):
        sl = slice(T * a, T * (a + 1))
        nc.scalar.activation(sc_sb[:, sl], cond_sb[:, sl], AF.Silu)

    h1_pool = pool("h1_pool")
    h1_sb = h1_pool.tile([128, 8 * T], BF16, name="h1_sb")

    gb1_pool = pool("gb1_pool")
    gb1_sb = gb1_pool.tile([128, 16 * T], BF16, name="gb1_sb")
    proj_gb(p1w_sb, p1b_sb, gb1_sb)

    aln1_tmp = pool("aln1_tmp")
    adaln(xT_sb, gb1_sb, h1_sb, aln1_tmp, st_ps, bc_ps, "a1")
    aln1_tmp.release()
    gb1_pool.release()

    # ---------------- qkv + chunked collective ----------------
    # 4 AllGather chunks, one per 4-head group: chunk c carries k^T feature
    # rows [256c:256c+256] and v columns [256c:256c+256], so attention on
    # head-pairs 2c,2c+1 can start while later chunks are still in flight.
    kv_pool = pool("kv_pool")
    kT_loc = kv_pool.tile([128, 8 * T], BF16, name="kT_loc")
    v_loc = kv_pool.tile([128, 4 * D], BF16, name="v_loc")

    NCH = 4
    kv_ins = [dram.tile([512, T], BF16, name=f"kv_in{c}") for c in range(NCH)]
    kv_outs = [dram.tile([GROUP, 512, T], BF16, name=f"kv_out{c}") for c in range(NCH)]

    for c in range(NCH):
        # k^T feature M-tiles for heads 4c..4c+3
        for ml in range(2):
            m = 8 + 2 * c + ml
            ps = mm_ps.tile([128, T], F32, name="kps", tag="mm")
            for k in range(8):
                nc.tensor.matmul(ps[:], qkvw_sb[:, 3072 * k + 128 * m: 3072 * k + 128 * (m + 1)],
                                 h1_sb[:, k * T:(k + 1) * T],
                                 start=(k == 0), stop=(k == 7))
            nc.vector.tensor_copy(kT_loc[:, (m - 8) * T:(m - 7) * T], ps[:])
        # v quarter c ([tokens, 256 features]), token M-tiles
        for mt in range(4):
            ps = mm_ps.tile([128, 256], F32, name="vps", tag="mm")
            for k in range(8):
                nc.tensor.matmul(
                    ps[:],
                    h1_sb[:, k * T + 128 * mt: k * T + 128 * (mt + 1)],
                    qkvw_sb[:, 3072 * k + 2048 + 256 * c: 3072 * k + 2048 + 256 * (c + 1)],
                    start=(k == 0), stop=(k == 7))
            nc.vector.tensor_copy(v_loc[:, 1024 * mt + 256 * c: 1024 * mt + 256 * (c + 1)], ps[:])
        # bounce writes + collective for this chunk
        for ml in range(2):
            nc.sync.dma_start(kv_ins[c][128 * ml:128 * (ml + 1), :],
                              kT_loc[:, (2 * c + ml) * T:(2 * c + ml + 1) * T])
        vdst = kv_ins[c][256:512, :].rearrange("r (two f) -> (r two) f", two=2)
        nc.sync.dma_start(vdst.rearrange("(m p) f -> p m f", m=4),
                          v_loc.rearrange("p (m f) -> p m f", m=4)[:, :, 256 * c:256 * (c + 1)])
        nc.gpsimd.collective_compute(
            "AllGather",
            ALU.bypass,
            replica_groups=[[0, 1, 2, 3], [4, 5, 6, 7]],
            ins=[kv_ins[c][:]],
            outs=[kv_outs[c][:]],
        )
    kv_pool.release()

    # right-side carries for the attention phase
    gb2_pool = pool("gb2_pool", side="right")
    gb2_sb = gb2_pool.tile([128, 16 * T], BF16, name="gb2_sb")
    oT_pool = pool("oT_pool", side="right")
    oT_sb = oT_pool.tile([128, 8 * T], BF16, name="oT_sb")
    q_pool = pool("q_pool", side="right")
    qT_sb = q_pool.tile([128, 8 * T], BF16, name="qT_sb")

    # q^T (feature M-tiles 0..7), overlaps with collective
    for m in range(8):
        ps = mm_ps.tile([128, T], F32, name="qps", tag="mm")
        for k in range(8):
            nc.tensor.matmul(ps[:], qkvw_sb[:, 3072 * k + 128 * m: 3072 * k + 128 * (m + 1)],
                             h1_sb[:, k * T:(k + 1) * T],
                             start=(k == 0), stop=(k == 7))
        nc.vector.tensor_copy(qT_sb[:, m * T:(m + 1) * T], ps[:])

    # gb2 projection, overlaps with collective
    proj_gb(p2w_sb, p2b_sb, gb2_sb)

    h1_pool.release()
    mm_ps.release()
    bc_ps.release()
    st_ps.release()
    qkvw_pool.release()
    proj_pool.release()
    cond_pool.release()

    # ---------------- attention ----------------
    att_pool = pool("att_pool")
    kT_full = att_pool.tile([128, 8 * S], BF16, name="kT_full")
    VW = DH + 1  # 65: per-head V columns + ones column (softmax denominator)
    vext = att_pool.tile([128, 16 * H * VW], BF16, name="vext")
    vext_v = vext.rearrange("p (c h m) -> p c h m", c=16, m=VW)
    nc.vector.memset(vext_v[:, :, :, DH:DH + 1], 1.0)

    def readback_chunk(c):
        for fl in range(2):
            f = 2 * c + fl
            for r in range(GROUP):
                nc.sync.dma_start(kT_full[:, 2048 * f + 512 * r: 2048 * f + 512 * (r + 1)],
                                  kv_outs[c][r, 128 * fl:128 * (fl + 1), :])
        for r in range(GROUP):
            vch = kv_outs[c][r, 256:512, :].rearrange("q (two f) -> (q two) f", two=2)
            for lc in range(4):
                c2 = 4 * r + lc
                src = vch[128 * lc:128 * (lc + 1), :].rearrange("t (h d) -> t h d", d=DH)
                # SWDGE queue: keeps vext readbacks off the sync DMA queues so
                # they don't serialize behind later chunks' waits
                nc.gpsimd.dma_start(vext_v[:, c2, 4 * c:4 * (c + 1), 0:DH], src)

    # emit all readbacks now, BEFORE any later weight loads: each chunk's
    # transfers wait on its AllGather semaphore, and weight bytes queued
    # ahead of them would delay the attention-critical data
    for c in range(NCH):
        readback_chunk(c)

    p_pool = pool("p_pool", bufs=4)
    norm_pool = pool("norm_pool", bufs=2)
    sc_ps = pool("sc_ps", bufs=3, space="PSUM")     # [128,1024] = 2 banks each
    o_ps_pool = pool("o_ps", bufs=2, space="PSUM")

    norm_pending = []

    def after_av(pv_hp, o_tiles):
        # Part A (DVE only): evict raw o^T + denominator (freeing o psum
        # quickly) and compute a fast-approx reciprocal; the PE-side broadcast
        # runs a pair later via flush_norm so the reciprocal chain never
        # stalls the in-order PE queue. Denominators live at partition 64
        # (engine partition starts must be 32-aligned).
        for hh in range(2):
            nc.vector.tensor_copy(oT_sb[64 * hh:64 * (hh + 1), pv_hp * T:(pv_hp + 1) * T],
                                  o_tiles[hh][0:DH, :])
            den = norm_pool.tile([128, T], F32, name="den", tag="den")
            nc.vector.tensor_copy(den[64:65, :], o_tiles[hh][DH:DH + 1, :])
            rec = norm_pool.tile([128, T], F32, name="rec", tag="rec", bufs=4)
            # full-tile op: the custom-DVE lowering needs a partition-0 start;
            # only row 64 is ever read downstream (other rows are junk/junk)
            nc.vector.reciprocal_approx_fast(out=rec[:], in_=den[:])
            norm_pending.append((pv_hp, hh, rec))

    def flush_norm():
        for (php, phh, rec) in norm_pending:
            rbt = sc_ps.tile([128, 2 * T], F32, name="rbt", tag="s")
            nc.tensor.matmul(rbt[0:64, 0:T], ones_all[64:65, :], rec[64:65, :],
                             start=True, stop=True)
            rb_sb = norm_pool.tile([128, T], BF16, name="rb_sb", tag="rbs")
            nc.vector.tensor_copy(rb_sb[64 * phh:64 * (phh + 1), :], rbt[0:64, 0:T])
            osl = oT_sb[64 * phh:64 * (phh + 1), php * T:(php + 1) * T]
            nc.vector.tensor_mul(osl, osl, rb_sb[64 * phh:64 * (phh + 1), :])
        norm_pending.clear()

    prev = None
    for hp in range(8):
        p_tiles = [p_pool.tile([128, 16 * T], BF16, name=f"pt{hh}", tag="p") for hh in range(2)]
        q_h = [qT_sb[64 * hh:64 * (hh + 1), hp * T:(hp + 1) * T] for hh in range(2)]
        o_tiles = None
        if prev is not None:
            o_tiles = [o_ps_pool.tile([128, T], F32, name="o_ps", tag="o") for _ in range(2)]
        # 8 groups: scores for chunks (2m2, 2m2+1) of both heads, interleaved
        # with 4 AV matmuls of the previous pair so PE work overlaps ACT exp.
        for m2 in range(8):
            scts = [sc_ps.tile([128, 1024], F32, name="sct", tag="s") for _ in range(2)]
            for half in range(2):
                m = 2 * m2 + half
                for hh in range(2):
                    rows = slice(64 * hh, 64 * (hh + 1))
                    nc.tensor.matmul(
                        scts[hh][:, 512 * half:512 * (half + 1)],
                        kT_full[rows, 2048 * hp + 128 * m: 2048 * hp + 128 * (m + 1)],
                        q_h[hh],
                        start=True, stop=True,
                        tile_position=(64 * hh, 0))
            if prev is not None:
                pv_tiles, pv_hp = prev
                for hh in range(2):
                    h = 2 * pv_hp + hh
                    for half in range(2):
                        cc = 2 * m2 + half
                        nc.tensor.matmul(
                            o_tiles[hh][0:VW, :],
                            vext[:, VW * (16 * cc + h): VW * (16 * cc + h) + VW],
                            pv_tiles[hh][:, cc * T:(cc + 1) * T],
                            start=(cc == 0), stop=(cc == 15))
            for hh in range(2):
                nc.scalar.activation(p_tiles[hh][:, 2 * m2 * T:(2 * m2 + 2) * T],
                                     scts[hh][:], AF.Exp)
        flush_norm()
        if prev is not None:
            after_av(prev[1], o_tiles)
        prev = (p_tiles, hp)

    # tail: AV + normalize for the last pair
    pv_tiles, pv_hp = prev
    o_tiles = [o_ps_pool.tile([128, T], F32, name="o_ps", tag="o") for _ in range(2)]
    for cc in range(16):
        for hh in range(2):
            h = 2 * pv_hp + hh
            nc.tensor.matmul(
                o_tiles[hh][0:VW, :],
                vext[:, VW * (16 * cc + h): VW * (16 * cc + h) + VW],
                pv_tiles[hh][:, cc * T:(cc + 1) * T],
                start=(cc == 0), stop=(cc == 15))
    flush_norm()
    after_av(pv_hp, o_tiles)
    flush_norm()

    o_ps_pool.release()
    sc_ps.release()
    norm_pool.release()
    p_pool.release()
    att_pool.release()

    # ---------------- attn_out + residual (in place into xT_sb) ----------
    mm_ps2 = pool("mm_ps2", bufs=2, space="PSUM")
    st_ps2 = pool("st_ps2", bufs=2, space="PSUM")
    bc_ps2 = pool("bc_ps2", bufs=1, space="PSUM")

    wo_pool = pool("wo_pool")
    wo_sb = wo_pool.tile([128, 8 * D], BF16, name="wo_sb")
    for a in range(8):
        nc.sync.dma_start(wo_sb[:, 1024 * a:1024 * (a + 1)], wo[128 * a:128 * (a + 1), :])

    for m in range(8):
        ps = mm_ps2.tile([128, T], F32, name="aops", tag="mm")
        for k in range(8):
            nc.tensor.matmul(ps[:], wo_sb[:, 1024 * k + 128 * m: 1024 * k + 128 * (m + 1)],
                             oT_sb[:, k * T:(k + 1) * T],
                             start=(k == 0), stop=(k == 7))
        nc.vector.tensor_add(xT_sb[:, m * T:(m + 1) * T], ps[:], xT_sb[:, m * T:(m + 1) * T])
    wo_pool.release()

    q_pool.release()
    oT_pool.release()

    # ---------------- AdaLN 2 ----------------
    g_pool = pool("g_pool")
    g_sb = g_pool.tile([128, 32 * T], BF16, name="g_sb")

    h2_pool = pool("h2_pool")
    h2_sb = h2_pool.tile([128, 8 * T], BF16, name="h2_sb")

    w1_pool = pool("w1_pool")
    w1_sb = w1_pool.tile([128, 8 * FF], BF16, name="w1_sb")
    for a in range(8):
        nc.sync.dma_start(w1_sb[:, 4096 * a:4096 * (a + 1)], w1[128 * a:128 * (a + 1), :])

    aln2_tmp = pool("aln2_tmp")
    adaln(xT_sb, gb2_sb, h2_sb, aln2_tmp, st_ps2, bc_ps2, "a2")
    aln2_tmp.release()
    gb2_pool.release()
    bc_ps2.release()
    st_ps2.release()

    # ---------------- FFN ----------------
    for m in range(32):
        ps = mm_ps2.tile([128, T], F32, name="f1ps", tag="mm")
        for k in range(8):
            nc.tensor.matmul(ps[:], w1_sb[:, 4096 * k + 128 * m: 4096 * k + 128 * (m + 1)],
                             h2_sb[:, k * T:(k + 1) * T],
                             start=(k == 0), stop=(k == 7))
        nc.scalar.activation(g_sb[:, m * T:(m + 1) * T], ps[:], AF.Gelu,
                             bias=b1_sb[:, m:m + 1], scale=1.0)
    w1_pool.release()
    h2_pool.release()
    mm_ps2.release()

    # ffn2: k-outer, stream w2 k-tiles; two m-halves so the first half's
    # evictions overlap the second half's matmuls
    w2_pool = pool("w2_pool", bufs=4)
    ff2_ps = pool("ff2_ps", bufs=1, space="PSUM")
    out_pool0 = pool("out_pool0")
    out_sb = out_pool0.tile([128, 8 * T], F32, name="out_sb")
    for half in range(2):
        o2 = [ff2_ps.tile([128, T], F32, name=f"ff2_{m}", tag=f"ff2_{m}") for m in range(4)]
        for k in range(32):
            w2t = w2_pool.tile([128, 512], BF16, name="w2t", tag="w2t")
            nc.sync.dma_start(w2t[:], w2[128 * k: 128 * (k + 1), 512 * half:512 * (half + 1)])
            for m in range(4):
                nc.tensor.matmul(o2[m][:], w2t[:, 128 * m: 128 * (m + 1)],
                                 g_sb[:, k * T:(k + 1) * T],
                                 start=(k == 0), stop=(k == 31))
        for m in range(4):
            gm = 4 * half + m
            nc.vector.scalar_tensor_tensor(out_sb[:, gm * T:(gm + 1) * T], o2[m][:],
                                           b2_sb[:, gm:gm + 1], xT_sb[:, gm * T:(gm + 1) * T],
                                           op0=ALU.add, op1=ALU.add)
        for a in range(4 * half, 4 * half + 4):
            nc.sync.dma_start(out_d[128 * a:128 * (a + 1), :], out_sb[:, T * a:T * (a + 1)])

    out_pool0.release()
    ff2_ps.release()
    w2_pool.release()
    g_pool.release()
    x_pool.release()
    small.release()
    work.release()
    const.release()
    dram.release()


def _bf16(a):
    return np.ascontiguousarray(a).astype(ml_dtypes.bfloat16)


def _prep_maps(x, cond, p1_w, p1_b, qkv_w, attn_out_w, p2_w, p2_b,
               ffn_w1, ffn_b1, ffn_w2, ffn_b2):
    x = np.asarray(x, np.float32)
    cond = np.asarray(cond, np.float32)
    qkv_mod = np.asarray(qkv_w, np.float32).copy()
    qkv_mod[:, :D] *= DH ** -0.5                      # fold 1/sqrt(d) into q
    p1b_mod = np.asarray(p1_b, np.float32).copy()
    p1b_mod[:D] += 1.0                                # fold AdaLN "+1" into gamma bias
    p2b_mod = np.asarray(p2_b, np.float32).copy()
    p2b_mod[:D] += 1.0

    shared = {
        "p1w": _bf16(p1_w),
        "p1b": np.ascontiguousarray(p1b_mod.reshape(16, 128).T, np.float32),
        "qkvw": _bf16(qkv_mod),
        "wo": _bf16(attn_out_w),
        "p2w": _bf16(p2_w),
        "p2b": np.ascontiguousarray(p2b_mod.reshape(16, 128).T, np.float32),
        "w1": _bf16(ffn_w1),
        "b1": np.ascontiguousarray(np.asarray(ffn_b1, np.float32).reshape(32, 128).T,
                                   np.float32),
        "w2": _bf16(ffn_w2),
        "b2": np.ascontiguousarray(np.asarray(ffn_b2, np.float32).reshape(8, 128).T,
                                   np.float32),
    }
    in_maps = []
    for core in range(NCORES):
        b, r = core // GROUP, core % GROUP
        sl = slice(T * r, T * (r + 1))
        m = dict(shared)
        m["xT"] = np.ascontiguousarray(x[b, sl, :].T, np.float32)
        m["condT"] = _bf16(cond[b, sl, :].T)
        in_maps.append(m)
    return in_maps


def _get_nc():
    if "nc" not in _CACHE:
        _CACHE["nc"] = _build()
    return _CACHE["nc"]


def _install_ntff_hook():
    """This image's antenv lacks axon_hooks; recreate it (see trn_boot.py)."""
    import sys, types, ctypes, contextlib

    if "antenv.axon_hooks" in sys.modules:
        return
    mod = types.ModuleType("antenv.axon_hooks")
    state = {"hook": None}
    mod.set_axon_ntff_profile_hook = lambda h: state.__setitem__("hook", h)
    mod.get_axon_ntff_profile_hook = lambda: state["hook"]
    sys.modules["antenv.axon_hooks"] = mod
    try:
        import antenv
        antenv.axon_hooks = mod
    except ImportError:
        pass

    so_path = "/opt/axon/libaxon_pjrt.so"
    if not os.path.exists(so_path):
        return
    lib = ctypes.CDLL(so_path)
    if not hasattr(lib, "axon_start_nrt_profile"):
        return
    lib.axon_start_nrt_profile.argtypes = [ctypes.POINTER(ctypes.c_int64), ctypes.c_size_t]
    lib.axon_start_nrt_profile.restype = ctypes.c_int64
    lib.axon_stop_nrt_profile.argtypes = [ctypes.c_char_p]
    lib.axon_stop_nrt_profile.restype = ctypes.c_int64

    @contextlib.contextmanager
    def _hook(output_dir, device_ids):
        import jax
        jax.devices()
        if device_ids:
            ids = (ctypes.c_int64 * len(device_ids))(*device_ids)
            rc = lib.axon_start_nrt_profile(ids, len(device_ids))
        else:
            rc = lib.axon_start_nrt_profile(None, 0)
        if rc != 0:
            raise RuntimeError(f"axon_start_nrt_profile rc={rc}")
        try:
            yield
        finally:
            n = lib.axon_stop_nrt_profile(str(output_dir).encode())
            print(f"ntff profile: {n} file(s) -> {output_dir}")

    mod.set_axon_ntff_profile_hook(_hook)


def run(in_maps, trace=False, **kw):
    if trace:
        _install_ntff_hook()
    nc = _get_nc()
    return run_bass_kernel_spmd(nc, in_maps, core_ids=list(range(NCORES)),
                                trace=trace, **kw)


def kernel(**inputs):
    in_maps = _prep_maps(**inputs)
    res = run(in_maps).results
    out = np.empty((B, S, D), np.float32)
    for core in range(NCORES):
        b, r = core // GROUP, core % GROUP
        out[b, T * r: T * (r + 1), :] = res[core]["out"].T
    return out


# revision 28
# speedup vs baseline: 1.1496x; 1.0533x over previous
# Distributed Bass kernel for nn_DecoderBlock (AdaLN decoder block) on 8 TRN2 cores.
#
# Sharding: core i -> (batch b = i//4, sequence quarter r = i%4, 512 tokens).
# Weights replicated (bf16). The only collective is a 4-rank AllGather of the
# local K^T / V slices per batch group (chunked 4x so attention starts early).
#
# Layout convention: every on-chip activation is stored transposed,
# [features(partitions), tokens(free)], so each linear y = h @ W uses the
# weight (in,out) directly as matmul lhsT and needs no on-chip transposes.
# Host pre-transposes/shards x and cond, folds 1/sqrt(d) into the q columns
# of qkv_w and the AdaLN "+1" into the gamma half of p1_b/p2_b.
#
# Perf notes vs the original baseline (~541-565us -> ~522us):
#  - tiny warm-up AllGather at t=0 absorbs the collective cold-start that
#    made the first real AllGather transfer ~3x slower than the rest
#    (67us -> 27us); first scores start at ~109us instead of ~143us
#  - input DMAs reordered (cond, x, p1w first); kv-chunk readbacks emitted
#    before any later weight loads so they never queue behind weight bytes
#  - adaln normalize applies in [128, 2*T] j-pairs (half the DVE dispatches)
#  - softmax denominators: reciprocal_approx_fast (full-tile: the custom-DVE
#    lowering needs a partition-0 start) instead of 3.3us vector.reciprocal
#  - score PSUM pool deepened to 3 slots; rb broadcasts borrow score slots
#  - x loaded once; the attention residual adds in place into the same tile

import os

os.environ.setdefault("MYCRO_LOCAL_CACHE", "1")

import numpy as np
import ml_dtypes

import concourse.bass as bass
import concourse.mybir as mybir
import concourse.tile as tile
from concourse import bacc
from concourse.bass_utils import run_bass_kernel_spmd

F32 = mybir.dt.float32
BF16 = mybir.dt.bfloat16
FP8 = mybir.dt.float8e4
AF = mybir.ActivationFunctionType
ALU = mybir.AluOpType

D = 1024        # d_model
DC = 512        # d_cond
H = 16          # heads
DH = 64         # head dim
FF = 4096       # ffn dim
T = 512         # tokens per core
S = 2048        # sequence length per batch
B = 2
NCORES = 8
GROUP = 4       # cores per batch group
EPS = 1e-5

_CACHE = {}


def _build():
    nc = bacc.Bacc(
        "TRN2",
        target_bir_lowering=False,
        debug=False,
        enable_asserts=False,
        num_devices=NCORES,
    )

    # ---- DRAM I/O ----
    xT = nc.dram_tensor("xT", [D, T], F32, kind="ExternalInput").ap()
    condT = nc.dram_tensor("condT", [DC, T], BF16, kind="ExternalInput").ap()
    p1w = nc.dram_tensor("p1w", [DC, 2 * D], BF16, kind="ExternalInput").ap()
    p1b = nc.dram_tensor("p1b", [128, 16], F32, kind="ExternalInput").ap()
    qkvw = nc.dram_tensor("qkvw", [D, 3 * D], BF16, kind="ExternalInput").ap()
    wo = nc.dram_tensor("wo", [D, D], BF16, kind="ExternalInput").ap()
    p2w = nc.dram_tensor("p2w", [DC, 2 * D], BF16, kind="ExternalInput").ap()
    p2b = nc.dram_tensor("p2b", [128, 16], F32, kind="ExternalInput").ap()
    w1 = nc.dram_tensor("w1", [D, FF], BF16, kind="ExternalInput").ap()
    b1 = nc.dram_tensor("b1", [128, 32], F32, kind="ExternalInput").ap()
    w2 = nc.dram_tensor("w2", [FF, D], BF16, kind="ExternalInput").ap()
    b2 = nc.dram_tensor("b2", [128, 8], F32, kind="ExternalInput").ap()
    out_d = nc.dram_tensor("out", [D, T], F32, kind="ExternalOutput").ap()

    with tile.TileContext(nc) as tc:
        _emit(nc, tc, xT, condT, p1w, p1b, qkvw, wo, p2w, p2b, w1, b1, w2, b2, out_d)

    nc.compile()
    return nc


def _emit(nc, tc, xT, condT, p1w, p1b, qkvw, wo, p2w, p2b, w1, b1, w2, b2, out_d):
    # Pool lifetimes follow a two-sided stack discipline (LIFO per side):
    # left = phase-nested pools, right = phase-crossing carries.
    def pool(name, bufs=1, space="SBUF", side=None):
        return tc.alloc_tile_pool(name=name, bufs=bufs, space=space, side=side)

    # ---------------- persistent pools ----------------
    const = pool("const")
    work = pool("work", bufs=4)            # [128,T] temporaries
    small = pool("small", bufs=4)          # [1,T] stats
    dram = pool("dram", bufs=1, space="DRAM")

    # right-side carry: x lives to the end; the attention residual adds into
    # it in place, so it doubles as x1.
    x_pool = pool("x_pool", side="right")
    xT_sb = x_pool.tile([128, 8 * T], F32, name="xT_sb")

    # ---------------- warm-up collective ----------------
    # The first AllGather of a NEFF runs far below link rate (ring/descriptor
    # cold start). Fire a tiny one immediately; it has no input dependencies
    # (the data is junk) and overlaps the input DMAs.
    wu_in = dram.tile([128, 16], BF16, name="wu_in")
    wu_out = dram.tile([GROUP, 128, 16], BF16, name="wu_out")
    nc.gpsimd.collective_compute(
        "AllGather",
        ALU.bypass,
        replica_groups=[[0, 1, 2, 3], [4, 5, 6, 7]],
        ins=[wu_in[:]],
        outs=[wu_out[:]],
    )

    # ---------------- constants ----------------
    ones_col_bf = const.tile([128, 1], BF16, name="ones_col_bf")
    nc.vector.memset(ones_col_bf[:], 1.0)
    ones_row_f = const.tile([1, 128], F32, name="ones_row_f")
    nc.vector.memset(ones_row_f[:], 1.0)
    eps_t = const.tile([1, 1], F32, name="eps_t")
    nc.vector.memset(eps_t[:], EPS)
    ones_all = const.tile([128, 64], F32, name="ones_all")
    nc.vector.memset(ones_all[:], 1.0)

    p1b_sb = const.tile([128, 16], F32, name="p1b_sb")
    nc.sync.dma_start(p1b_sb[:], p1b)
    p2b_sb = const.tile([128, 16], F32, name="p2b_sb")
    nc.sync.dma_start(p2b_sb[:], p2b)
    b1_sb = const.tile([128, 32], F32, name="b1_sb")
    nc.sync.dma_start(b1_sb[:], b1)
    b2_sb = const.tile([128, 8], F32, name="b2_sb")
    nc.sync.dma_start(b2_sb[:], b2)

    # ---------------- input loads, priority order ----------------
    # cond gates silu -> gb1; x gates the adaln1 stats; p1w gates gb1;
    # qkvw is needed ~20us in (kv chunk matmuls); p2w only at ~45us.
    cond_pool = pool("cond_pool")
    cond_sb = cond_pool.tile([128, 4 * T], BF16, name="cond_sb")
    for a in range(4):
        nc.sync.dma_start(cond_sb[:, T * a:T * (a + 1)], condT[128 * a:128 * (a + 1), :])
    sc_sb = cond_pool.tile([128, 4 * T], BF16, name="sc_sb")

    for a in range(8):
        nc.sync.dma_start(xT_sb[:, T * a:T * (a + 1)], xT[128 * a:128 * (a + 1), :])

    proj_pool = pool("proj_pool")
    p1w_sb = proj_pool.tile([128, 4 * 2048], BF16, name="p1w_sb")
    for a in range(4):
        nc.sync.dma_start(p1w_sb[:, 2048 * a:2048 * (a + 1)], p1w[128 * a:128 * (a + 1), :])

    qkvw_pool = pool("qkvw_pool")
    qkvw_sb = qkvw_pool.tile([128, 8 * 3072], BF16, name="qkvw_sb")
    for a in range(8):
        nc.sync.dma_start(qkvw_sb[:, 3072 * a:3072 * (a + 1)], qkvw[128 * a:128 * (a + 1), :])

    p2w_sb = proj_pool.tile([128, 4 * 2048], BF16, name="p2w_sb")
    for a in range(4):
        nc.sync.dma_start(p2w_sb[:, 2048 * a:2048 * (a + 1)], p2w[128 * a:128 * (a + 1), :])

    # PSUM pools for the pre-phase: evictions (2) + stats (2) + broadcast (4)
    st_ps = pool("st_ps", bufs=2, space="PSUM")     # [1,T] stats accumulators
    bc_ps = pool("bc_ps", bufs=1, space="PSUM")     # [128, 4T] mu/rs broadcast
    mm_ps = pool("mm_ps", bufs=2, space="PSUM")     # matmul eviction banks

    # ---------------- helpers ----------------
    def adaln(src_f32, gb_sb, h_sb, tmp_pool, stp, bcp, prefix):
        """src_f32: [128, 8*T] f32 ([D, T] transposed); writes h_sb bf16.

        LayerNorm stats via PE ones-matmuls, then normalize+affine applied
        in [128, 2*T] j-pairs (gamma tiles 0..7 and beta tiles 8..15 of
        gb_sb are each contiguous, so pairs slice cleanly).
        """
        src_bf = tmp_pool.tile([128, 8 * T], BF16, name=f"{prefix}src_bf", tag="src_bf")
        sq = tmp_pool.tile([128, 8 * T], BF16, name=f"{prefix}sq", tag="sq")
        for hv in range(2):
            hs = slice(4 * T * hv, 4 * T * (hv + 1))
            nc.vector.tensor_copy(src_bf[:, hs], src_f32[:, hs])
            nc.vector.tensor_mul(sq[:, hs], src_bf[:, hs], src_bf[:, hs])

        sums = stp.tile([1, T], F32, name="sums", tag="st")
        for j in range(8):
            nc.tensor.matmul(sums[:], ones_col_bf[:], src_bf[:, j * T:(j + 1) * T],
                             start=(j == 0), stop=(j == 7))
        sumsq = stp.tile([1, T], F32, name="sumsq", tag="st")
        for j in range(8):
            nc.tensor.matmul(sumsq[:], ones_col_bf[:], sq[:, j * T:(j + 1) * T],
                             start=(j == 0), stop=(j == 7))

        mu = small.tile([1, T], F32, name="mu", tag="sm")
        nc.vector.tensor_scalar_mul(mu[:], sums[:], 1.0 / D)
        musq = small.tile([1, T], F32, name="musq", tag="sm")
        nc.vector.tensor_mul(musq[:], mu[:], mu[:])
        var = small.tile([1, T], F32, name="var", tag="sm")
        nc.vector.scalar_tensor_tensor(var[:], sumsq[:], 1.0 / D, musq[:],
                                       op0=ALU.mult, op1=ALU.subtract)
        lnv = small.tile([1, T], F32, name="lnv", tag="sm")
        nc.scalar.activation(lnv[:], var[:], AF.Ln, bias=eps_t[:], scale=1.0)
        rs = small.tile([1, T], F32, name="rs", tag="sm")
        nc.scalar.activation(rs[:], lnv[:], AF.Exp, scale=-0.5)

        # broadcast mu and rs to 128 partitions, each repeated twice along
        # the free dim so the j-pair ops below can consume them directly
        mrs_b = bcp.tile([128, 4 * T], F32, name="mrs_b", tag="bc")
        for r in range(2):
            nc.tensor.matmul(mrs_b[:, r * T:(r + 1) * T], ones_row_f[:], mu[:],
                             start=True, stop=True)
            nc.tensor.matmul(mrs_b[:, (2 + r) * T:(3 + r) * T], ones_row_f[:], rs[:],
                             start=True, stop=True)
        mrs_bs = tmp_pool.tile([128, 4 * T], BF16, name=f"{prefix}mrs_bs", tag="mrs")
        nc.vector.tensor_copy(mrs_bs[:], mrs_b[:])
        mu2 = mrs_bs[:, 0:2 * T]
        rs2 = mrs_bs[:, 2 * T:4 * T]

        for jp in range(0, 8, 2):
            sl = slice(jp * T, (jp + 2) * T)
            bsl = slice((8 + jp) * T, (10 + jp) * T)
            t1 = work.tile([128, 2 * T], BF16, name="t1", tag="wk2")
            nc.vector.tensor_sub(t1[:], src_bf[:, sl], mu2)
            t2 = work.tile([128, 2 * T], BF16, name="t2", tag="wk2")
            nc.vector.tensor_mul(t2[:], t1[:], rs2)
            t3 = work.tile([128, 2 * T], BF16, name="t3", tag="wk2")
            nc.vector.tensor_mul(t3[:], t2[:], gb_sb[:, sl])           # *(1+gamma)
            nc.vector.tensor_add(h_sb[:, sl], t3[:], gb_sb[:, bsl])

    def proj_gb(w_sb, b_sb, gb_sb):
        """gb^T = (silu(cond) @ W + b)^T : 16 M-tiles of [128, T]."""
        for m in range(16):
            ps = mm_ps.tile([128, T], F32, name="gbps", tag="mm")
            for k in range(4):
                nc.tensor.matmul(ps[:], w_sb[:, 2048 * k + 128 * m: 2048 * k + 128 * (m + 1)],
                                 sc_sb[:, k * T:(k + 1) * T],
                                 start=(k == 0), stop=(k == 3))
            nc.vector.tensor_scalar_add(gb_sb[:, m * T:(m + 1) * T], ps[:],
                                        b_sb[:, m:m + 1])

    # ---------------- AdaLN 1 ----------------
    for a in range(4):
        sl = slice(T * a, T * (a + 1))
        nc.scalar.activation(sc_sb[:, sl], cond_sb[:, sl], AF.Silu)

    h1_pool = pool("h1_pool")
    h1_sb = h1_pool.tile([128, 8 * T], BF16, name="h1_sb")

    gb1_pool = pool("gb1_pool")
    gb1_sb = gb1_pool.tile([128, 16 * T], BF16, name="gb1_sb")
    proj_gb(p1w_sb, p1b_sb, gb1_sb)

    aln1_tmp = pool("aln1_tmp")
    adaln(xT_sb, gb1_sb, h1_sb, aln1_tmp, st_ps, bc_ps, "a1")
    aln1_tmp.release()
    gb1_pool.release()

    # ---------------- qkv + chunked collective ----------------
    # 4 AllGather chunks, one per 4-head group: chunk c carries k^T feature
    # rows [256c:256c+256] and v columns [256c:256c+256], so attention on
    # head-pairs 2c,2c+1 can start while later chunks are still in flight.
    kv_pool = pool("kv_pool")
    kT_loc = kv_pool.tile([128, 8 * T], BF16, name="kT_loc")
    v_loc = kv_pool.tile([128, 4 * D], BF16, name="v_loc")

    NCH = 4
    kv_ins = [dram.tile([512, T], BF16, name=f"kv_in{c}") for c in range(NCH)]
    kv_outs = [dram.tile([GROUP, 512, T], BF16, name=f"kv_out{c}") for c in range(NCH)]

    for c in range(NCH):
        # k^T feature M-tiles for heads 4c..4c+3
        for ml in range(2):
            m = 8 + 2 * c + ml
            ps = mm_ps.tile([128, T], F32, name="kps", tag="mm")
            for k in range(8):
                nc.tensor.matmul(ps[:], qkvw_sb[:, 3072 * k + 128 * m: 3072 * k + 128 * (m + 1)],
                                 h1_sb[:, k * T:(k + 1) * T],
                                 start=(k == 0), stop=(k == 7))
            nc.vector.tensor_copy(kT_loc[:, (m - 8) * T:(m - 7) * T], ps[:])
        # v quarter c ([tokens, 256 features]), token M-tiles
        for mt in range(4):
            ps = mm_ps.tile([128, 256], F32, name="vps", tag="mm")
            for k in range(8):
                nc.tensor.matmul(
                    ps[:],
                    h1_sb[:, k * T + 128 * mt: k * T + 128 * (mt + 1)],
                    qkvw_sb[:, 3072 * k + 2048 + 256 * c: 3072 * k + 2048 + 256 * (c + 1)],
                    start=(k == 0), stop=(k == 7))
            nc.vector.tensor_copy(v_loc[:, 1024 * mt + 256 * c: 1024 * mt + 256 * (c + 1)], ps[:])
        # bounce writes + collective for this chunk
        for ml in range(2):
            nc.sync.dma_start(kv_ins[c][128 * ml:128 * (ml + 1), :],
                              kT_loc[:, (2 * c + ml) * T:(2 * c + ml + 1) * T])
        vdst = kv_ins[c][256:512, :].rearrange("r (two f) -> (r two) f", two=2)
        nc.sync.dma_start(vdst.rearrange("(m p) f -> p m f", m=4),
                          v_loc.rearrange("p (m f) -> p m f", m=4)[:, :, 256 * c:256 * (c + 1)])
        nc.gpsimd.collective_compute(
            "AllGather",
            ALU.bypass,
            replica_groups=[[0, 1, 2, 3], [4, 5, 6, 7]],
            ins=[kv_ins[c][:]],
            outs=[kv_outs[c][:]],
        )
    kv_pool.release()

    # right-side carries for the attention phase
    gb2_pool = pool("gb2_pool", side="right")
    gb2_sb = gb2_pool.tile([128, 16 * T], BF16, name="gb2_sb")
    oT_pool = pool("oT_pool", side="right")
    oT_sb = oT_pool.tile([128, 8 * T], BF16, name="oT_sb")
    q_pool = pool("q_pool", side="right")
    qT_sb = q_pool.tile([128, 8 * T], BF16, name="qT_sb")

    # q^T (feature M-tiles 0..7), overlaps with collective
    for m in range(8):
        ps = mm_ps.tile([128, T], F32, name="qps", tag="mm")
        for k in range(8):
            nc.tensor.matmul(ps[:], qkvw_sb[:, 3072 * k + 128 * m: 3072 * k + 128 * (m + 1)],
                             h1_sb[:, k * T:(k + 1) * T],
                             start=(k == 0), stop=(k == 7))
        nc.vector.tensor_copy(qT_sb[:, m * T:(m + 1) * T], ps[:])

    # gb2 projection, overlaps with collective
    proj_gb(p2w_sb, p2b_sb, gb2_sb)

    h1_pool.release()
    mm_ps.release()
    bc_ps.release()
    st_ps.release()
    qkvw_pool.release()
    proj_pool.release()
    cond_pool.release()

    # ---------------- attention ----------------
    att_pool = pool("att_pool")
    kT_full = att_pool.tile([128, 8 * S], BF16, name="kT_full")
    VW = DH + 1  # 65: per-head V columns + ones column (softmax denominator)
    vext = att_pool.tile([128, 16 * H * VW], BF16, name="vext")
    vext_v = vext.rearrange("p (c h m) -> p c h m", c=16, m=VW)
    nc.vector.memset(vext_v[:, :, :, DH:DH + 1], 1.0)

    def readback_chunk(c):
        for fl in range(2):
            f = 2 * c + fl
            for r in range(GROUP):
                nc.sync.dma_start(kT_full[:, 2048 * f + 512 * r: 2048 * f + 512 * (r + 1)],
                                  kv_outs[c][r, 128 * fl:128 * (fl + 1), :])
        for r in range(GROUP):
            vch = kv_outs[c][r, 256:512, :].rearrange("q (two f) -> (q two) f", two=2)
            for lc in range(4):
                c2 = 4 * r + lc
                src = vch[128 * lc:128 * (lc + 1), :].rearrange("t (h d) -> t h d", d=DH)
                # SWDGE queue: keeps vext readbacks off the sync DMA queues so
                # they don't serialize behind later chunks' waits
                nc.gpsimd.dma_start(vext_v[:, c2, 4 * c:4 * (c + 1), 0:DH], src)

    # emit all readbacks now, BEFORE any later weight loads: each chunk's
    # transfers wait on its AllGather semaphore, and weight bytes queued
    # ahead of them would delay the attention-critical data
    for c in range(NCH):
        readback_chunk(c)

    p_pool = pool("p_pool", bufs=4)
    norm_pool = pool("norm_pool", bufs=2)
    sc_ps = pool("sc_ps", bufs=3, space="PSUM")     # [128,1024] = 2 banks each
    o_ps_pool = pool("o_ps", bufs=2, space="PSUM")

    # Persistent den/rec ping-pong pairs: both heads' denominators live in
    # ONE tile, at partitions 64 (head 0) and 96 (head 1), so a single
    # fast-reciprocal + a single sel-matrix matmul + one [128,T] multiply
    # normalize a whole head pair (was: 2 matmuls + 2 casts + 2 muls).
    # den rows are primed to 1.0 once, so the junk rows stay finite through
    # the full-tile reciprocal and contribute sel=0 * finite = 0.
    den_t = [norm_pool.tile([128, T], F32, name=f"den{i}", tag=f"den{i}", bufs=1)
             for i in range(2)]
    rec_t = [norm_pool.tile([128, T], F32, name=f"rec{i}", tag=f"rec{i}", bufs=1)
             for i in range(2)]
    for i in range(2):
        nc.vector.memset(den_t[i][:], 1.0)
    # sel_b: contraction rows 64/96 select rec rows 64/96 into output halves
    sel_b = att_pool.tile([128, 128], F32, name="sel_b")
    nc.vector.memset(sel_b[:], 0.0)
    nc.vector.memset(sel_b[64:65, 0:64], 1.0)
    nc.vector.memset(sel_b[96:97, 64:128], 1.0)

    norm_pending = []

    def after_av(pv_hp, o_tiles):
        # Part A (DVE only): evict raw o^T + denominators (freeing o psum
        # quickly) and compute one fast-approx reciprocal for both heads; the
        # PE-side broadcast runs a pair later via flush_norm so the
        # reciprocal chain never stalls the in-order PE queue.
        den = den_t[pv_hp % 2]
        rec = rec_t[pv_hp % 2]
        for hh in range(2):
            nc.vector.tensor_copy(oT_sb[64 * hh:64 * (hh + 1), pv_hp * T:(pv_hp + 1) * T],
                                  o_tiles[hh][0:DH, :])
            nc.vector.tensor_copy(den[64 + 32 * hh:65 + 32 * hh, :],
                                  o_tiles[hh][DH:DH + 1, :])
        # full-tile op: the custom-DVE lowering needs a partition-0 start
        nc.vector.reciprocal_approx_fast(out=rec[:], in_=den[:])
        norm_pending.append((pv_hp, rec))

    def flush_norm():
        for (php, rec) in norm_pending:
            rbt = sc_ps.tile([128, 2 * T], F32, name="rbt", tag="s")
            nc.tensor.matmul(rbt[:, 0:T], sel_b[64:128, :], rec[64:128, :],
                             start=True, stop=True)
            rb_sb = norm_pool.tile([128, T], BF16, name="rb_sb", tag="rbs")
            nc.vector.tensor_copy(rb_sb[:], rbt[:, 0:T])
            osl = oT_sb[:, php * T:(php + 1) * T]
            nc.vector.tensor_mul(osl, osl, rb_sb[:])
        norm_pending.clear()

    prev = None
    for hp in range(8):
        p_tiles = [p_pool.tile([128, 16 * T], BF16, name=f"pt{hh}", tag="p") for hh in range(2)]
        q_h = [qT_sb[64 * hh:64 * (hh + 1), hp * T:(hp + 1) * T] for hh in range(2)]
        o_tiles = None
        if prev is not None:
            o_tiles = [o_ps_pool.tile([128, T], F32, name="o_ps", tag="o") for _ in range(2)]
        # 8 groups: scores for chunks (2m2, 2m2+1) of both heads, interleaved
        # with 4 AV matmuls of the previous pair so PE work overlaps ACT exp.
        for m2 in range(8):
            scts = [sc_ps.tile([128, 1024], F32, name="sct", tag="s") for _ in range(2)]
            for half in range(2):
                m = 2 * m2 + half
                for hh in range(2):
                    rows = slice(64 * hh, 64 * (hh + 1))
                    nc.tensor.matmul(
                        scts[hh][:, 512 * half:512 * (half + 1)],
                        kT_full[rows, 2048 * hp + 128 * m: 2048 * hp + 128 * (m + 1)],
                        q_h[hh],
                        start=True, stop=True)
            if prev is not None:
                pv_tiles, pv_hp = prev
                for hh in range(2):
                    h = 2 * pv_hp + hh
                    for half in range(2):
                        cc = 2 * m2 + half
                        nc.tensor.matmul(
                            o_tiles[hh][0:VW, :],
                            vext[:, VW * (16 * cc + h): VW * (16 * cc + h) + VW],
                            pv_tiles[hh][:, cc * T:(cc + 1) * T],
                            start=(cc == 0), stop=(cc == 15))
            for hh in range(2):
                nc.scalar.activation(p_tiles[hh][:, 2 * m2 * T:(2 * m2 + 2) * T],
                                     scts[hh][:], AF.Exp)
        flush_norm()
        if prev is not None:
            after_av(prev[1], o_tiles)
        prev = (p_tiles, hp)

    # tail: AV + normalize for the last pair
    pv_tiles, pv_hp = prev
    o_tiles = [o_ps_pool.tile([128, T], F32, name="o_ps", tag="o") for _ in range(2)]
    for cc in range(16):
        for hh in range(2):
            h = 2 * pv_hp + hh
            nc.tensor.matmul(
                o_tiles[hh][0:VW, :],
                vext[:, VW * (16 * cc + h): VW * (16 * cc + h) + VW],
                pv_tiles[hh][:, cc * T:(cc + 1) * T],
                start=(cc == 0), stop=(cc == 15))
    flush_norm()
    after_av(pv_hp, o_tiles)
    flush_norm()

    o_ps_pool.release()
    sc_ps.release()
    norm_pool.release()
    p_pool.release()
    att_pool.release()

    # ---------------- attn_out + residual (in place into xT_sb) ----------
    mm_ps2 = pool("mm_ps2", bufs=2, space="PSUM")
    st_ps2 = pool("st_ps2", bufs=2, space="PSUM")
    bc_ps2 = pool("bc_ps2", bufs=1, space="PSUM")

    wo_pool = pool("wo_pool")
    wo_sb = wo_pool.tile([128, 8 * D], BF16, name="wo_sb")
    for a in range(8):
        nc.sync.dma_start(wo_sb[:, 1024 * a:1024 * (a + 1)], wo[128 * a:128 * (a + 1), :])

    for m in range(8):
        ps = mm_ps2.tile([128, T], F32, name="aops", tag="mm")
        for k in range(8):
            nc.tensor.matmul(ps[:], wo_sb[:, 1024 * k + 128 * m: 1024 * k + 128 * (m + 1)],
                             oT_sb[:, k * T:(k + 1) * T],
                             start=(k == 0), stop=(k == 7))
        nc.vector.tensor_add(xT_sb[:, m * T:(m + 1) * T], ps[:], xT_sb[:, m * T:(m + 1) * T])
    wo_pool.release()

    q_pool.release()
    oT_pool.release()

    # ---------------- AdaLN 2 ----------------
    g_pool = pool("g_pool")
    g_sb = g_pool.tile([128, 32 * T], BF16, name="g_sb")

    h2_pool = pool("h2_pool")
    h2_sb = h2_pool.tile([128, 8 * T], BF16, name="h2_sb")

    w1_pool = pool("w1_pool")
    w1_sb = w1_pool.tile([128, 8 * FF], BF16, name="w1_sb")
    for a in range(8):
        nc.sync.dma_start(w1_sb[:, 4096 * a:4096 * (a + 1)], w1[128 * a:128 * (a + 1), :])

    aln2_tmp = pool("aln2_tmp")
    adaln(xT_sb, gb2_sb, h2_sb, aln2_tmp, st_ps2, bc_ps2, "a2")
    aln2_tmp.release()
    gb2_pool.release()
    bc_ps2.release()
    st_ps2.release()

    # ---------------- FFN ----------------
    for m in range(32):
        ps = mm_ps2.tile([128, T], F32, name="f1ps", tag="mm")
        for k in range(8):
            nc.tensor.matmul(ps[:], w1_sb[:, 4096 * k + 128 * m: 4096 * k + 128 * (m + 1)],
                             h2_sb[:, k * T:(k + 1) * T],
                             start=(k == 0), stop=(k == 7))
        nc.scalar.activation(g_sb[:, m * T:(m + 1) * T], ps[:], AF.Gelu,
                             bias=b1_sb[:, m:m + 1], scale=1.0)
    w1_pool.release()
    h2_pool.release()
    mm_ps2.release()

    # ffn2: k-outer, stream w2 k-tiles; two m-halves so the first half's
    # evictions overlap the second half's matmuls
    w2_pool = pool("w2_pool", bufs=4)
    ff2_ps = pool("ff2_ps", bufs=1, space="PSUM")
    out_pool0 = pool("out_pool0")
    out_sb = out_pool0.tile([128, 8 * T], F32, name="out_sb")
    for half in range(2):
        o2 = [ff2_ps.tile([128, T], F32, name=f"ff2_{m}", tag=f"ff2_{m}") for m in range(4)]
        for k in range(32):
            w2t = w2_pool.tile([128, 512], BF16, name="w2t", tag="w2t")
            nc.sync.dma_start(w2t[:], w2[128 * k: 128 * (k + 1), 512 * half:512 * (half + 1)])
            for m in range(4):
                nc.tensor.matmul(o2[m][:], w2t[:, 128 * m: 128 * (m + 1)],
                                 g_sb[:, k * T:(k + 1) * T],
                                 start=(k == 0), stop=(k == 31))
        for m in range(4):
            gm = 4 * half + m
            nc.vector.scalar_tensor_tensor(out_sb[:, gm * T:(gm + 1) * T], o2[m][:],
                                           b2_sb[:, gm:gm + 1], xT_sb[:, gm * T:(gm + 1) * T],
                                           op0=ALU.add, op1=ALU.add)
        for a in range(4 * half, 4 * half + 4):
            nc.sync.dma_start(out_d[128 * a:128 * (a + 1), :], out_sb[:, T * a:T * (a + 1)])

    out_pool0.release()
    ff2_ps.release()
    w2_pool.release()
    g_pool.release()
    x_pool.release()
    small.release()
    work.release()
    const.release()
    dram.release()


def _bf16(a):
    return np.ascontiguousarray(a).astype(ml_dtypes.bfloat16)


def _prep_maps(x, cond, p1_w, p1_b, qkv_w, attn_out_w, p2_w, p2_b,
               ffn_w1, ffn_b1, ffn_w2, ffn_b2):
    x = np.asarray(x, np.float32)
    cond = np.asarray(cond, np.float32)
    qkv_mod = np.asarray(qkv_w, np.float32).copy()
    qkv_mod[:, :D] *= DH ** -0.5                      # fold 1/sqrt(d) into q
    p1b_mod = np.asarray(p1_b, np.float32).copy()
    p1b_mod[:D] += 1.0                                # fold AdaLN "+1" into gamma bias
    p2b_mod = np.asarray(p2_b, np.float32).copy()
    p2b_mod[:D] += 1.0

    shared = {
        "p1w": _bf16(p1_w),
        "p1b": np.ascontiguousarray(p1b_mod.reshape(16, 128).T, np.float32),
        "qkvw": _bf16(qkv_mod),
        "wo": _bf16(attn_out_w),
        "p2w": _bf16(p2_w),
        "p2b": np.ascontiguousarray(p2b_mod.reshape(16, 128).T, np.float32),
        "w1": _bf16(ffn_w1),
        "b1": np.ascontiguousarray(np.asarray(ffn_b1, np.float32).reshape(32, 128).T,
                                   np.float32),
        "w2": _bf16(ffn_w2),
        "b2": np.ascontiguousarray(np.asarray(ffn_b2, np.float32).reshape(8, 128).T,
                                   np.float32),
    }
    in_maps = []
    for core in range(NCORES):
        b, r = core // GROUP, core % GROUP
        sl = slice(T * r, T * (r + 1))
        m = dict(shared)
        m["xT"] = np.ascontiguousarray(x[b, sl, :].T, np.float32)
        m["condT"] = _bf16(cond[b, sl, :].T)
        in_maps.append(m)
    return in_maps


def _get_nc():
    if "nc" not in _CACHE:
        _CACHE["nc"] = _build()
    return _CACHE["nc"]


def _install_ntff_hook():
    """This image's antenv lacks axon_hooks; recreate it (see trn_boot.py)."""
    import sys, types, ctypes, contextlib

    if "antenv.axon_hooks" in sys.modules:
        return
    mod = types.ModuleType("antenv.axon_hooks")
    state = {"hook": None}
    mod.set_axon_ntff_profile_hook = lambda h: state.__setitem__("hook", h)
    mod.get_axon_ntff_profile_hook = lambda: state["hook"]
    sys.modules["antenv.axon_hooks"] = mod
    try:
        import antenv
        antenv.axon_hooks = mod
    except ImportError:
        pass

    so_path = "/opt/axon/libaxon_pjrt.so"
    if not os.path.exists(so_path):
        return
    lib = ctypes.CDLL(so_path)
    if not hasattr(lib, "axon_start_nrt_profile"):
        return
    lib.axon_start_nrt_profile.argtypes = [ctypes.POINTER(ctypes.c_int64), ctypes.c_size_t]
    lib.axon_start_nrt_profile.restype = ctypes.c_int64
    lib.axon_stop_nrt_profile.argtypes = [ctypes.c_char_p]
    lib.axon_stop_nrt_profile.restype = ctypes.c_int64

    @contextlib.contextmanager
    def _hook(output_dir, device_ids):
        import jax
        jax.devices()
        if device_ids:
            ids = (ctypes.c_int64 * len(device_ids))(*device_ids)
            rc = lib.axon_start_nrt_profile(ids, len(device_ids))
        else:
            rc = lib.axon_start_nrt_profile(None, 0)
        if rc != 0:
            raise RuntimeError(f"axon_start_nrt_profile rc={rc}")
        try:
            yield
        finally:
            n = lib.axon_stop_nrt_profile(str(output_dir).encode())
            print(f"ntff profile: {n} file(s) -> {output_dir}")

    mod.set_axon_ntff_profile_hook(_hook)


def run(in_maps, trace=False, **kw):
    if trace:
        _install_ntff_hook()
    nc = _get_nc()
    return run_bass_kernel_spmd(nc, in_maps, core_ids=list(range(NCORES)),
                                trace=trace, **kw)


def kernel(**inputs):
    in_maps = _prep_maps(**inputs)
    res = run(in_maps).results
    out = np.empty((B, S, D), np.float32)
    for core in range(NCORES):
        b, r = core // GROUP, core % GROUP
        out[b, T * r: T * (r + 1), :] = res[core]["out"].T
    return out


# revision 29
# speedup vs baseline: 1.1807x; 1.0270x over previous
# Distributed Bass kernel for nn_DecoderBlock (AdaLN decoder block) on 8 TRN2 cores.
#
# Sharding: core i -> (batch b = i//4, sequence quarter r = i%4, 512 tokens).
# Weights replicated (bf16). The only collective is a 4-rank AllGather of the
# local K^T / V slices per batch group (chunked 4x so attention starts early).
#
# Layout convention: every on-chip activation is stored transposed,
# [features(partitions), tokens(free)], so each linear y = h @ W uses the
# weight (in,out) directly as matmul lhsT and needs no on-chip transposes.
# Host pre-transposes/shards x and cond, folds 1/sqrt(d) into the q columns
# of qkv_w and the AdaLN "+1" into the gamma half of p1_b/p2_b.
#
# Perf notes vs the original baseline (~541-565us -> ~522us):
#  - tiny warm-up AllGather at t=0 absorbs the collective cold-start that
#    made the first real AllGather transfer ~3x slower than the rest
#    (67us -> 27us); first scores start at ~109us instead of ~143us
#  - input DMAs reordered (cond, x, p1w first); kv-chunk readbacks emitted
#    before any later weight loads so they never queue behind weight bytes
#  - adaln normalize applies in [128, 2*T] j-pairs (half the DVE dispatches)
#  - softmax denominators: reciprocal_approx_fast (full-tile: the custom-DVE
#    lowering needs a partition-0 start) instead of 3.3us vector.reciprocal
#  - score PSUM pool deepened to 3 slots; rb broadcasts borrow score slots
#  - x loaded once; the attention residual adds in place into the same tile

import os

os.environ.setdefault("MYCRO_LOCAL_CACHE", "1")

import numpy as np
import ml_dtypes

import concourse.bass as bass
import concourse.mybir as mybir
import concourse.tile as tile
from concourse import bacc
from concourse.bass_utils import run_bass_kernel_spmd

F32 = mybir.dt.float32
BF16 = mybir.dt.bfloat16
FP8 = mybir.dt.float8e4
AF = mybir.ActivationFunctionType
ALU = mybir.AluOpType

D = 1024        # d_model
DC = 512        # d_cond
H = 16          # heads
DH = 64         # head dim
FF = 4096       # ffn dim
T = 512         # tokens per core
S = 2048        # sequence length per batch
B = 2
NCORES = 8
GROUP = 4       # cores per batch group
EPS = 1e-5

_CACHE = {}


def _build():
    nc = bacc.Bacc(
        "TRN2",
        target_bir_lowering=False,
        debug=False,
        enable_asserts=False,
        num_devices=NCORES,
    )

    # ---- DRAM I/O ----
    xT = nc.dram_tensor("xT", [D, T], F32, kind="ExternalInput").ap()
    condT = nc.dram_tensor("condT", [DC, T], BF16, kind="ExternalInput").ap()
    p1w = nc.dram_tensor("p1w", [DC, 2 * D], BF16, kind="ExternalInput").ap()
    p1b = nc.dram_tensor("p1b", [128, 16], F32, kind="ExternalInput").ap()
    qkvw = nc.dram_tensor("qkvw", [D, 3 * D], BF16, kind="ExternalInput").ap()
    wo = nc.dram_tensor("wo", [D, D], BF16, kind="ExternalInput").ap()
    p2w = nc.dram_tensor("p2w", [DC, 2 * D], BF16, kind="ExternalInput").ap()
    p2b = nc.dram_tensor("p2b", [128, 16], F32, kind="ExternalInput").ap()
    w1 = nc.dram_tensor("w1", [D, FF], BF16, kind="ExternalInput").ap()
    b1 = nc.dram_tensor("b1", [128, 32], F32, kind="ExternalInput").ap()
    w2 = nc.dram_tensor("w2", [FF, D], BF16, kind="ExternalInput").ap()
    b2 = nc.dram_tensor("b2", [128, 8], F32, kind="ExternalInput").ap()
    out_d = nc.dram_tensor("out", [D, T], F32, kind="ExternalOutput").ap()

    with tile.TileContext(nc) as tc:
        _emit(nc, tc, xT, condT, p1w, p1b, qkvw, wo, p2w, p2b, w1, b1, w2, b2, out_d)

    nc.compile()
    return nc


def _emit(nc, tc, xT, condT, p1w, p1b, qkvw, wo, p2w, p2b, w1, b1, w2, b2, out_d):
    # Pool lifetimes follow a two-sided stack discipline (LIFO per side):
    # left = phase-nested pools, right = phase-crossing carries.
    def pool(name, bufs=1, space="SBUF", side=None):
        return tc.alloc_tile_pool(name=name, bufs=bufs, space=space, side=side)

    # ---------------- persistent pools ----------------
    const = pool("const")
    work = pool("work", bufs=4)            # [128,T] temporaries
    small = pool("small", bufs=4)          # [1,T] stats
    dram = pool("dram", bufs=1, space="DRAM")

    # right-side carry: x lives to the end; the attention residual adds into
    # it in place, so it doubles as x1.
    x_pool = pool("x_pool", side="right")
    xT_sb = x_pool.tile([128, 8 * T], F32, name="xT_sb")

    # ---------------- warm-up collective ----------------
    # The first AllGather of a NEFF runs far below link rate (ring/descriptor
    # cold start). Fire a tiny one immediately; it has no input dependencies
    # (the data is junk) and overlaps the input DMAs.
    wu_in = dram.tile([128, 16], BF16, name="wu_in")
    wu_out = dram.tile([GROUP, 128, 16], BF16, name="wu_out")
    nc.gpsimd.collective_compute(
        "AllGather",
        ALU.bypass,
        replica_groups=[[0, 1, 2, 3], [4, 5, 6, 7]],
        ins=[wu_in[:]],
        outs=[wu_out[:]],
    )

    # ---------------- constants ----------------
    ones_col_bf = const.tile([128, 1], BF16, name="ones_col_bf")
    nc.vector.memset(ones_col_bf[:], 1.0)
    ones_row_f = const.tile([1, 128], F32, name="ones_row_f")
    nc.vector.memset(ones_row_f[:], 1.0)
    eps_t = const.tile([1, 1], F32, name="eps_t")
    nc.vector.memset(eps_t[:], EPS)
    ones_all = const.tile([128, 64], F32, name="ones_all")
    nc.vector.memset(ones_all[:], 1.0)

    p1b_sb = const.tile([128, 16], F32, name="p1b_sb")
    nc.sync.dma_start(p1b_sb[:], p1b)
    p2b_sb = const.tile([128, 16], F32, name="p2b_sb")
    nc.sync.dma_start(p2b_sb[:], p2b)
    b1_sb = const.tile([128, 32], F32, name="b1_sb")
    nc.sync.dma_start(b1_sb[:], b1)
    b2_sb = const.tile([128, 8], F32, name="b2_sb")
    nc.sync.dma_start(b2_sb[:], b2)

    # ---------------- input loads, priority order ----------------
    # cond gates silu -> gb1; x gates the adaln1 stats; p1w gates gb1;
    # qkvw is needed ~20us in (kv chunk matmuls); p2w only at ~45us.
    cond_pool = pool("cond_pool")
    cond_sb = cond_pool.tile([128, 4 * T], BF16, name="cond_sb")
    for a in range(4):
        nc.sync.dma_start(cond_sb[:, T * a:T * (a + 1)], condT[128 * a:128 * (a + 1), :])
    sc_sb = cond_pool.tile([128, 4 * T], BF16, name="sc_sb")

    for a in range(8):
        nc.sync.dma_start(xT_sb[:, T * a:T * (a + 1)], xT[128 * a:128 * (a + 1), :])

    proj_pool = pool("proj_pool")
    p1w_sb = proj_pool.tile([128, 4 * 2048], BF16, name="p1w_sb")
    for half in range(2):
        for a in range(4):
            nc.sync.dma_start(
                p1w_sb[:, 2048 * a + 1024 * half: 2048 * a + 1024 * (half + 1)],
                p1w[128 * a:128 * (a + 1), 1024 * half:1024 * (half + 1)])

    qkvw_pool = pool("qkvw_pool")
    qkvw_sb = qkvw_pool.tile([128, 8 * 3072], BF16, name="qkvw_sb")
    for a in range(8):
        nc.sync.dma_start(qkvw_sb[:, 3072 * a:3072 * (a + 1)], qkvw[128 * a:128 * (a + 1), :])

    p2w_sb = proj_pool.tile([128, 4 * 2048], BF16, name="p2w_sb")
    for a in range(4):
        nc.sync.dma_start(p2w_sb[:, 2048 * a:2048 * (a + 1)], p2w[128 * a:128 * (a + 1), :])

    # PSUM pools for the pre-phase: evictions (3) + stats (2) + broadcast (2)
    st_ps = pool("st_ps", bufs=2, space="PSUM")     # [1,T] stats accumulators
    bc_ps = pool("bc_ps", bufs=1, space="PSUM")     # [128, 2T] mu/rs broadcast
    mm_ps = pool("mm_ps", bufs=3, space="PSUM")     # matmul eviction banks

    # ---------------- helpers ----------------
    def adaln(src_f32, gb_sb, h_sb, tmp_pool, stp, bcp, prefix):
        """src_f32: [128, 8*T] f32 ([D, T] transposed); writes h_sb bf16.

        LayerNorm stats via PE ones-matmuls, then normalize+affine applied
        in [128, 2*T] j-pairs (gamma tiles 0..7 and beta tiles 8..15 of
        gb_sb are each contiguous, so pairs slice cleanly).
        """
        src_bf = tmp_pool.tile([128, 8 * T], BF16, name=f"{prefix}src_bf", tag="src_bf")
        sq = tmp_pool.tile([128, 8 * T], BF16, name=f"{prefix}sq", tag="sq")
        for hv in range(2):
            hs = slice(4 * T * hv, 4 * T * (hv + 1))
            nc.vector.tensor_copy(src_bf[:, hs], src_f32[:, hs])
            nc.vector.tensor_mul(sq[:, hs], src_bf[:, hs], src_bf[:, hs])

        sums = stp.tile([1, T], F32, name="sums", tag="st")
        for j in range(8):
            nc.tensor.matmul(sums[:], ones_col_bf[:], src_bf[:, j * T:(j + 1) * T],
                             start=(j == 0), stop=(j == 7))
        sumsq = stp.tile([1, T], F32, name="sumsq", tag="st")
        for j in range(8):
            nc.tensor.matmul(sumsq[:], ones_col_bf[:], sq[:, j * T:(j + 1) * T],
                             start=(j == 0), stop=(j == 7))

        mu = small.tile([1, T], F32, name="mu", tag="sm")
        nc.vector.tensor_scalar_mul(mu[:], sums[:], 1.0 / D)
        musq = small.tile([1, T], F32, name="musq", tag="sm")
        nc.vector.tensor_mul(musq[:], mu[:], mu[:])
        var = small.tile([1, T], F32, name="var", tag="sm")
        nc.vector.scalar_tensor_tensor(var[:], sumsq[:], 1.0 / D, musq[:],
                                       op0=ALU.mult, op1=ALU.subtract)
        lnv = small.tile([1, T], F32, name="lnv", tag="sm")
        nc.scalar.activation(lnv[:], var[:], AF.Ln, bias=eps_t[:], scale=1.0)
        rs = small.tile([1, T], F32, name="rs", tag="sm")
        nc.scalar.activation(rs[:], lnv[:], AF.Exp, scale=-0.5)

        # broadcast mu and rs to 128 partitions with one matmul each; the
        # x2 repeat the j-pair ops need comes from duplicated casts (cheaper
        # than extra PE broadcasts, and the psum tile shrinks to 2 banks)
        mrs_b = bcp.tile([128, 2 * T], F32, name="mrs_b", tag="bc")
        nc.tensor.matmul(mrs_b[:, 0:T], ones_row_f[:], mu[:], start=True, stop=True)
        nc.tensor.matmul(mrs_b[:, T:2 * T], ones_row_f[:], rs[:], start=True, stop=True)
        mrs_bs = tmp_pool.tile([128, 4 * T], BF16, name=f"{prefix}mrs_bs", tag="mrs")
        for r in range(2):
            nc.vector.tensor_copy(mrs_bs[:, r * T:(r + 1) * T], mrs_b[:, 0:T])
            nc.vector.tensor_copy(mrs_bs[:, (2 + r) * T:(3 + r) * T], mrs_b[:, T:2 * T])
        mu2 = mrs_bs[:, 0:2 * T]
        rs2 = mrs_bs[:, 2 * T:4 * T]

        for jp in range(0, 8, 2):
            sl = slice(jp * T, (jp + 2) * T)
            bsl = slice((8 + jp) * T, (10 + jp) * T)
            t1 = work.tile([128, 2 * T], BF16, name="t1", tag="wk2")
            nc.vector.tensor_sub(t1[:], src_bf[:, sl], mu2)
            t2 = work.tile([128, 2 * T], BF16, name="t2", tag="wk2")
            nc.vector.tensor_mul(t2[:], t1[:], rs2)
            t3 = work.tile([128, 2 * T], BF16, name="t3", tag="wk2")
            nc.vector.tensor_mul(t3[:], t2[:], gb_sb[:, sl])           # *(1+gamma)
            nc.vector.tensor_add(h_sb[:, sl], t3[:], gb_sb[:, bsl])

    def proj_gb(w_sb, b_sb, gb_sb):
        """gb^T = (silu(cond) @ W + b)^T : 16 M-tiles of [128, T]."""
        for m in range(16):
            ps = mm_ps.tile([128, T], F32, name="gbps", tag="mm")
            for k in range(4):
                nc.tensor.matmul(ps[:], w_sb[:, 2048 * k + 128 * m: 2048 * k + 128 * (m + 1)],
                                 sc_sb[:, k * T:(k + 1) * T],
                                 start=(k == 0), stop=(k == 3))
            nc.vector.tensor_scalar_add(gb_sb[:, m * T:(m + 1) * T], ps[:],
                                        b_sb[:, m:m + 1])

    # ---------------- AdaLN 1 ----------------
    for a in range(4):
        sl = slice(T * a, T * (a + 1))
        nc.scalar.activation(sc_sb[:, sl], cond_sb[:, sl], AF.Silu)

    h1_pool = pool("h1_pool")
    h1_sb = h1_pool.tile([128, 8 * T], BF16, name="h1_sb")

    gb1_pool = pool("gb1_pool")
    gb1_sb = gb1_pool.tile([128, 16 * T], BF16, name="gb1_sb")
    proj_gb(p1w_sb, p1b_sb, gb1_sb)

    aln1_tmp = pool("aln1_tmp")
    adaln(xT_sb, gb1_sb, h1_sb, aln1_tmp, st_ps, bc_ps, "a1")
    aln1_tmp.release()
    gb1_pool.release()

    # ---------------- qkv + chunked collective ----------------
    # 4 AllGather chunks, one per 4-head group: chunk c carries k^T feature
    # rows [256c:256c+256] and v columns [256c:256c+256], so attention on
    # head-pairs 2c,2c+1 can start while later chunks are still in flight.
    kv_pool = pool("kv_pool")
    kT_loc = kv_pool.tile([128, 8 * T], BF16, name="kT_loc")
    v_loc = kv_pool.tile([128, 4 * D], BF16, name="v_loc")

    NCH = 4
    kv_ins = [dram.tile([512, T], BF16, name=f"kv_in{c}") for c in range(NCH)]
    kv_outs = [dram.tile([GROUP, 512, T], BF16, name=f"kv_out{c}") for c in range(NCH)]

    for c in range(NCH):
        # k^T feature M-tiles for heads 4c..4c+3
        for ml in range(2):
            m = 8 + 2 * c + ml
            ps = mm_ps.tile([128, T], F32, name="kps", tag="mm")
            for k in range(8):
                nc.tensor.matmul(ps[:], qkvw_sb[:, 3072 * k + 128 * m: 3072 * k + 128 * (m + 1)],
                                 h1_sb[:, k * T:(k + 1) * T],
                                 start=(k == 0), stop=(k == 7))
            nc.vector.tensor_copy(kT_loc[:, (m - 8) * T:(m - 7) * T], ps[:])
        # v quarter c ([tokens, 256 features]), token M-tiles
        for mt in range(4):
            ps = mm_ps.tile([128, 256], F32, name="vps", tag="mm")
            for k in range(8):
                nc.tensor.matmul(
                    ps[:],
                    h1_sb[:, k * T + 128 * mt: k * T + 128 * (mt + 1)],
                    qkvw_sb[:, 3072 * k + 2048 + 256 * c: 3072 * k + 2048 + 256 * (c + 1)],
                    start=(k == 0), stop=(k == 7))
            nc.vector.tensor_copy(v_loc[:, 1024 * mt + 256 * c: 1024 * mt + 256 * (c + 1)], ps[:])
        # bounce writes + collective for this chunk
        for ml in range(2):
            nc.sync.dma_start(kv_ins[c][128 * ml:128 * (ml + 1), :],
                              kT_loc[:, (2 * c + ml) * T:(2 * c + ml + 1) * T])
        vdst = kv_ins[c][256:512, :].rearrange("r (two f) -> (r two) f", two=2)
        nc.sync.dma_start(vdst.rearrange("(m p) f -> p m f", m=4),
                          v_loc.rearrange("p (m f) -> p m f", m=4)[:, :, 256 * c:256 * (c + 1)])
        nc.gpsimd.collective_compute(
            "AllGather",
            ALU.bypass,
            replica_groups=[[0, 1, 2, 3], [4, 5, 6, 7]],
            ins=[kv_ins[c][:]],
            outs=[kv_outs[c][:]],
        )
    kv_pool.release()

    # right-side carries for the attention phase
    gb2_pool = pool("gb2_pool", side="right")
    gb2_sb = gb2_pool.tile([128, 16 * T], BF16, name="gb2_sb")
    oT_pool = pool("oT_pool", side="right")
    oT_sb = oT_pool.tile([128, 8 * T], BF16, name="oT_sb")
    q_pool = pool("q_pool", side="right")
    qT_sb = q_pool.tile([128, 8 * T], BF16, name="qT_sb")

    # q^T (feature M-tiles 0..7), overlaps with collective
    for m in range(8):
        ps = mm_ps.tile([128, T], F32, name="qps", tag="mm")
        for k in range(8):
            nc.tensor.matmul(ps[:], qkvw_sb[:, 3072 * k + 128 * m: 3072 * k + 128 * (m + 1)],
                             h1_sb[:, k * T:(k + 1) * T],
                             start=(k == 0), stop=(k == 7))
        nc.vector.tensor_copy(qT_sb[:, m * T:(m + 1) * T], ps[:])

    # gb2 projection, overlaps with collective
    proj_gb(p2w_sb, p2b_sb, gb2_sb)

    h1_pool.release()
    mm_ps.release()
    bc_ps.release()
    st_ps.release()
    qkvw_pool.release()
    proj_pool.release()
    cond_pool.release()

    # ---------------- attention ----------------
    att_pool = pool("att_pool")
    kT_full = att_pool.tile([128, 8 * S], BF16, name="kT_full")
    VW = DH + 1  # 65: per-head V columns + ones column (softmax denominator)
    vext = att_pool.tile([128, 16 * H * VW], BF16, name="vext")
    vext_v = vext.rearrange("p (c h m) -> p c h m", c=16, m=VW)
    nc.vector.memset(vext_v[:, :, :, DH:DH + 1], 1.0)

    def readback_chunk(c):
        for fl in range(2):
            f = 2 * c + fl
            for r in range(GROUP):
                nc.sync.dma_start(kT_full[:, 2048 * f + 512 * r: 2048 * f + 512 * (r + 1)],
                                  kv_outs[c][r, 128 * fl:128 * (fl + 1), :])
        for r in range(GROUP):
            vch = kv_outs[c][r, 256:512, :].rearrange("q (two f) -> (q two) f", two=2)
            for lc in range(4):
                c2 = 4 * r + lc
                src = vch[128 * lc:128 * (lc + 1), :].rearrange("t (h d) -> t h d", d=DH)
                # SWDGE queue: keeps vext readbacks off the sync DMA queues so
                # they don't serialize behind later chunks' waits
                nc.gpsimd.dma_start(vext_v[:, c2, 4 * c:4 * (c + 1), 0:DH], src)

    # emit all readbacks now, BEFORE any later weight loads: each chunk's
    # transfers wait on its AllGather semaphore, and weight bytes queued
    # ahead of them would delay the attention-critical data
    for c in range(NCH):
        readback_chunk(c)

    p_pool = pool("p_pool", bufs=4)
    norm_pool = pool("norm_pool", bufs=2)
    sc_ps = pool("sc_ps", bufs=3, space="PSUM")     # [128,1024] = 2 banks each
    o_ps_pool = pool("o_ps", bufs=2, space="PSUM")

    # Persistent den/rec ping-pong pairs: both heads' denominators live in
    # ONE tile, at partitions 64 (head 0) and 96 (head 1), so a single
    # fast-reciprocal + a single sel-matrix matmul + one [128,T] multiply
    # normalize a whole head pair (was: 2 matmuls + 2 casts + 2 muls).
    # den rows are primed to 1.0 once, so the junk rows stay finite through
    # the full-tile reciprocal and contribute sel=0 * finite = 0.
    den_t = [norm_pool.tile([128, T], F32, name=f"den{i}", tag=f"den{i}", bufs=1)
             for i in range(2)]
    rec_t = [norm_pool.tile([128, T], F32, name=f"rec{i}", tag=f"rec{i}", bufs=1)
             for i in range(2)]
    for i in range(2):
        nc.vector.memset(den_t[i][:], 1.0)
    # sel_b: contraction rows 64/96 select rec rows 64/96 into output halves
    sel_b = att_pool.tile([128, 128], F32, name="sel_b")
    nc.vector.memset(sel_b[:], 0.0)
    nc.vector.memset(sel_b[64:65, 0:64], 1.0)
    nc.vector.memset(sel_b[96:97, 64:128], 1.0)

    norm_pending = []

    def after_av(pv_hp, o_tiles):
        # Part A (DVE only): evict raw o^T + denominators (freeing o psum
        # quickly) and compute one fast-approx reciprocal for both heads; the
        # PE-side broadcast runs a pair later via flush_norm so the
        # reciprocal chain never stalls the in-order PE queue.
        den = den_t[pv_hp % 2]
        rec = rec_t[pv_hp % 2]
        for hh in range(2):
            nc.vector.tensor_copy(oT_sb[64 * hh:64 * (hh + 1), pv_hp * T:(pv_hp + 1) * T],
                                  o_tiles[hh][0:DH, :])
            nc.vector.tensor_copy(den[64 + 32 * hh:65 + 32 * hh, :],
                                  o_tiles[hh][DH:DH + 1, :])
        # full-tile op: the custom-DVE lowering needs a partition-0 start
        nc.vector.reciprocal_approx_fast(out=rec[:], in_=den[:])
        norm_pending.append((pv_hp, rec))

    def flush_norm():
        for (php, rec) in norm_pending:
            rbt = sc_ps.tile([128, 2 * T], F32, name="rbt", tag="s")
            nc.tensor.matmul(rbt[:, 0:T], sel_b[64:128, :], rec[64:128, :],
                             start=True, stop=True)
            rb_sb = norm_pool.tile([128, T], BF16, name="rb_sb", tag="rbs")
            nc.vector.tensor_copy(rb_sb[:], rbt[:, 0:T])
            osl = oT_sb[:, php * T:(php + 1) * T]
            nc.vector.tensor_mul(osl, osl, rb_sb[:])
        norm_pending.clear()

    prev = None
    for hp in range(8):
        p_tiles = [p_pool.tile([128, 16 * T], BF16, name=f"pt{hh}", tag="p") for hh in range(2)]
        q_h = [qT_sb[64 * hh:64 * (hh + 1), hp * T:(hp + 1) * T] for hh in range(2)]
        o_tiles = None
        if prev is not None:
            o_tiles = [o_ps_pool.tile([128, T], F32, name="o_ps", tag="o") for _ in range(2)]
        # 8 groups: scores for chunks (2m2, 2m2+1) of both heads, interleaved
        # with 4 AV matmuls of the previous pair so PE work overlaps ACT exp.
        for m2 in range(8):
            scts = [sc_ps.tile([128, 1024], F32, name="sct", tag="s") for _ in range(2)]
            for half in range(2):
                m = 2 * m2 + half
                for hh in range(2):
                    rows = slice(64 * hh, 64 * (hh + 1))
                    nc.tensor.matmul(
                        scts[hh][:, 512 * half:512 * (half + 1)],
                        kT_full[rows, 2048 * hp + 128 * m: 2048 * hp + 128 * (m + 1)],
                        q_h[hh],
                        start=True, stop=True)
            if prev is not None:
                pv_tiles, pv_hp = prev
                for hh in range(2):
                    h = 2 * pv_hp + hh
                    for half in range(2):
                        cc = 2 * m2 + half
                        nc.tensor.matmul(
                            o_tiles[hh][0:VW, :],
                            vext[:, VW * (16 * cc + h): VW * (16 * cc + h) + VW],
                            pv_tiles[hh][:, cc * T:(cc + 1) * T],
                            start=(cc == 0), stop=(cc == 15))
            for hh in range(2):
                nc.scalar.activation(p_tiles[hh][:, 2 * m2 * T:(2 * m2 + 2) * T],
                                     scts[hh][:], AF.Exp)
        flush_norm()
        if prev is not None:
            after_av(prev[1], o_tiles)
        prev = (p_tiles, hp)

    # tail: AV + normalize for the last pair
    pv_tiles, pv_hp = prev
    o_tiles = [o_ps_pool.tile([128, T], F32, name="o_ps", tag="o") for _ in range(2)]
    for cc in range(16):
        for hh in range(2):
            h = 2 * pv_hp + hh
            nc.tensor.matmul(
                o_tiles[hh][0:VW, :],
                vext[:, VW * (16 * cc + h): VW * (16 * cc + h) + VW],
                pv_tiles[hh][:, cc * T:(cc + 1) * T],
                start=(cc == 0), stop=(cc == 15))
    flush_norm()
    after_av(pv_hp, o_tiles)
    flush_norm()

    o_ps_pool.release()
    sc_ps.release()
    norm_pool.release()
    p_pool.release()
    att_pool.release()

    # ---------------- attn_out + residual (in place into xT_sb) ----------
    mm_ps2 = pool("mm_ps2", bufs=3, space="PSUM")
    st_ps2 = pool("st_ps2", bufs=2, space="PSUM")
    bc_ps2 = pool("bc_ps2", bufs=1, space="PSUM")

    wo_pool = pool("wo_pool")
    wo_sb = wo_pool.tile([128, 8 * D], BF16, name="wo_sb")
    for a in range(8):
        nc.sync.dma_start(wo_sb[:, 1024 * a:1024 * (a + 1)], wo[128 * a:128 * (a + 1), :])

    for m in range(8):
        ps = mm_ps2.tile([128, T], F32, name="aops", tag="mm")
        for k in range(8):
            nc.tensor.matmul(ps[:], wo_sb[:, 1024 * k + 128 * m: 1024 * k + 128 * (m + 1)],
                             oT_sb[:, k * T:(k + 1) * T],
                             start=(k == 0), stop=(k == 7))
        nc.vector.tensor_add(xT_sb[:, m * T:(m + 1) * T], ps[:], xT_sb[:, m * T:(m + 1) * T])
    wo_pool.release()

    q_pool.release()
    oT_pool.release()

    # ---------------- AdaLN 2 ----------------
    g_pool = pool("g_pool")
    g_sb = g_pool.tile([128, 32 * T], BF16, name="g_sb")

    h2_pool = pool("h2_pool")
    h2_sb = h2_pool.tile([128, 8 * T], BF16, name="h2_sb")

    w1_pool = pool("w1_pool")
    w1_sb = w1_pool.tile([128, 8 * FF], BF16, name="w1_sb")
    for a in range(8):
        nc.sync.dma_start(w1_sb[:, 4096 * a:4096 * (a + 1)], w1[128 * a:128 * (a + 1), :])

    aln2_tmp = pool("aln2_tmp")
    adaln(xT_sb, gb2_sb, h2_sb, aln2_tmp, st_ps2, bc_ps2, "a2")
    aln2_tmp.release()
    gb2_pool.release()
    bc_ps2.release()
    st_ps2.release()

    # ---------------- FFN ----------------
    for m in range(32):
        ps = mm_ps2.tile([128, T], F32, name="f1ps", tag="mm")
        for k in range(8):
            nc.tensor.matmul(ps[:], w1_sb[:, 4096 * k + 128 * m: 4096 * k + 128 * (m + 1)],
                             h2_sb[:, k * T:(k + 1) * T],
                             start=(k == 0), stop=(k == 7))
        nc.scalar.activation(g_sb[:, m * T:(m + 1) * T], ps[:], AF.Gelu,
                             bias=b1_sb[:, m:m + 1], scale=1.0)
    w1_pool.release()
    h2_pool.release()
    mm_ps2.release()

    # ffn2: k-outer, stream w2 k-tiles; two m-halves so the first half's
    # evictions overlap the second half's matmuls
    w2_pool = pool("w2_pool", bufs=8)
    ff2_ps = pool("ff2_ps", bufs=1, space="PSUM")
    out_pool0 = pool("out_pool0")
    out_sb = out_pool0.tile([128, 8 * T], F32, name="out_sb")
    for half in range(2):
        o2 = [ff2_ps.tile([128, T], F32, name=f"ff2_{m}", tag=f"ff2_{m}") for m in range(4)]
        for k in range(32):
            w2t = w2_pool.tile([128, 512], BF16, name="w2t", tag="w2t")
            nc.sync.dma_start(w2t[:], w2[128 * k: 128 * (k + 1), 512 * half:512 * (half + 1)])
            for m in range(4):
                nc.tensor.matmul(o2[m][:], w2t[:, 128 * m: 128 * (m + 1)],
                                 g_sb[:, k * T:(k + 1) * T],
                                 start=(k == 0), stop=(k == 31))
        for m in range(4):
            gm = 4 * half + m
            nc.vector.scalar_tensor_tensor(out_sb[:, gm * T:(gm + 1) * T], o2[m][:],
                                           b2_sb[:, gm:gm + 1], xT_sb[:, gm * T:(gm + 1) * T],
                                           op0=ALU.add, op1=ALU.add)
        for a in range(4 * half, 4 * half + 4):
            nc.sync.dma_start(out_d[128 * a:128 * (a + 1), :], out_sb[:, T * a:T * (a + 1)])

    out_pool0.release()
    ff2_ps.release()
    w2_pool.release()
    g_pool.release()
    x_pool.release()
    small.release()
    work.release()
    const.release()
    dram.release()


def _bf16(a):
    return np.ascontiguousarray(a).astype(ml_dtypes.bfloat16)


def _prep_maps(x, cond, p1_w, p1_b, qkv_w, attn_out_w, p2_w, p2_b,
               ffn_w1, ffn_b1, ffn_w2, ffn_b2):
    x = np.asarray(x, np.float32)
    cond = np.asarray(cond, np.float32)
    qkv_mod = np.asarray(qkv_w, np.float32).copy()
    qkv_mod[:, :D] *= DH ** -0.5                      # fold 1/sqrt(d) into q
    p1b_mod = np.asarray(p1_b, np.float32).copy()
    p1b_mod[:D] += 1.0                                # fold AdaLN "+1" into gamma bias
    p2b_mod = np.asarray(p2_b, np.float32).copy()
    p2b_mod[:D] += 1.0

    shared = {
        "p1w": _bf16(p1_w),
        "p1b": np.ascontiguousarray(p1b_mod.reshape(16, 128).T, np.float32),
        "qkvw": _bf16(qkv_mod),
        "wo": _bf16(attn_out_w),
        "p2w": _bf16(p2_w),
        "p2b": np.ascontiguousarray(p2b_mod.reshape(16, 128).T, np.float32),
        "w1": _bf16(ffn_w1),
        "b1": np.ascontiguousarray(np.asarray(ffn_b1, np.float32).reshape(32, 128).T,
                                   np.float32),
        "w2": _bf16(ffn_w2),
        "b2": np.ascontiguousarray(np.asarray(ffn_b2, np.float32).reshape(8, 128).T,
                                   np.float32),
    }
    in_maps = []
    for core in range(NCORES):
        b, r = core // GROUP, core % GROUP
        sl = slice(T * r, T * (r + 1))
        m = dict(shared)
        m["xT"] = np.ascontiguousarray(x[b, sl, :].T, np.float32)
        m["condT"] = _bf16(cond[b, sl, :].T)
        in_maps.append(m)
    return in_maps


def _get_nc():
    if "nc" not in _CACHE:
        _CACHE["nc"] = _build()
    return _CACHE["nc"]


def _install_ntff_hook():
    """This image's antenv lacks axon_hooks; recreate it (see trn_boot.py)."""
    import sys, types, ctypes, contextlib

    if "antenv.axon_hooks" in sys.modules:
        return
    mod = types.ModuleType("antenv.axon_hooks")
    state = {"hook": None}
    mod.set_axon_ntff_profile_hook = lambda h: state.__setitem__("hook", h)
    mod.get_axon_ntff_profile_hook = lambda: state["hook"]
    sys.modules["antenv.axon_hooks"] = mod
    try:
        import antenv
        antenv.axon_hooks = mod
    except ImportError:
        pass

    so_path = "/opt/axon/libaxon_pjrt.so"
    if not os.path.exists(so_path):
        return
    lib = ctypes.CDLL(so_path)
    if not hasattr(lib, "axon_start_nrt_profile"):
        return
    lib.axon_start_nrt_profile.argtypes = [ctypes.POINTER(ctypes.c_int64), ctypes.c_size_t]
    lib.axon_start_nrt_profile.restype = ctypes.c_int64
    lib.axon_stop_nrt_profile.argtypes = [ctypes.c_char_p]
    lib.axon_stop_nrt_profile.restype = ctypes.c_int64

    @contextlib.contextmanager
    def _hook(output_dir, device_ids):
        import jax
        jax.devices()
        if device_ids:
            ids = (ctypes.c_int64 * len(device_ids))(*device_ids)
            rc = lib.axon_start_nrt_profile(ids, len(device_ids))
        else:
            rc = lib.axon_start_nrt_profile(None, 0)
        if rc != 0:
            raise RuntimeError(f"axon_start_nrt_profile rc={rc}")
        try:
            yield
        finally:
            n = lib.axon_stop_nrt_profile(str(output_dir).encode())
            print(f"ntff profile: {n} file(s) -> {output_dir}")

    mod.set_axon_ntff_profile_hook(_hook)


def run(in_maps, trace=False, **kw):
    if trace:
        _install_ntff_hook()
    nc = _get_nc()
    return run_bass_kernel_spmd(nc, in_maps, core_ids=list(range(NCORES)),
                                trace=trace, **kw)


def kernel(**inputs):
    in_maps = _prep_maps(**inputs)
    res = run(in_maps).results
    out = np.empty((B, S, D), np.float32)
    for core in range(NCORES):
        b, r = core // GROUP, core % GROUP
        out[b, T * r: T * (r + 1), :] = res[core]["out"].T
    return out


# revision 30
# speedup vs baseline: 1.1829x; 1.0019x over previous
# Distributed Bass kernel for nn_DecoderBlock (AdaLN decoder block) on 8 TRN2 cores.
#
# Sharding: core i -> (batch b = i//4, sequence quarter r = i%4, 512 tokens).
# Weights replicated (bf16). The only collective is a 4-rank AllGather of the
# local K^T / V slices per batch group (chunked 4x so attention starts early).
#
# Layout convention: every on-chip activation is stored transposed,
# [features(partitions), tokens(free)], so each linear y = h @ W uses the
# weight (in,out) directly as matmul lhsT and needs no on-chip transposes.
# Host pre-transposes/shards x and cond, folds 1/sqrt(d) into the q columns
# of qkv_w and the AdaLN "+1" into the gamma half of p1_b/p2_b.
#
# Perf notes vs the original baseline (~541-565us -> ~522us):
#  - tiny warm-up AllGather at t=0 absorbs the collective cold-start that
#    made the first real AllGather transfer ~3x slower than the rest
#    (67us -> 27us); first scores start at ~109us instead of ~143us
#  - input DMAs reordered (cond, x, p1w first); kv-chunk readbacks emitted
#    before any later weight loads so they never queue behind weight bytes
#  - adaln normalize applies in [128, 2*T] j-pairs (half the DVE dispatches)
#  - softmax denominators: reciprocal_approx_fast (full-tile: the custom-DVE
#    lowering needs a partition-0 start) instead of 3.3us vector.reciprocal
#  - score PSUM pool deepened to 3 slots; rb broadcasts borrow score slots
#  - x loaded once; the attention residual adds in place into the same tile

import os

os.environ.setdefault("MYCRO_LOCAL_CACHE", "1")

import numpy as np
import ml_dtypes

import concourse.bass as bass
import concourse.mybir as mybir
import concourse.tile as tile
from concourse import bacc
from concourse.bass_utils import run_bass_kernel_spmd

F32 = mybir.dt.float32
BF16 = mybir.dt.bfloat16
FP8 = mybir.dt.float8e4
AF = mybir.ActivationFunctionType
ALU = mybir.AluOpType

D = 1024        # d_model
DC = 512        # d_cond
H = 16          # heads
DH = 64         # head dim
FF = 4096       # ffn dim
T = 512         # tokens per core
S = 2048        # sequence length per batch
B = 2
NCORES = 8
GROUP = 4       # cores per batch group
EPS = 1e-5

_CACHE = {}


def _build():
    nc = bacc.Bacc(
        "TRN2",
        target_bir_lowering=False,
        debug=False,
        enable_asserts=False,
        num_devices=NCORES,
    )

    # ---- DRAM I/O ----
    xT = nc.dram_tensor("xT", [D, T], F32, kind="ExternalInput").ap()
    condT = nc.dram_tensor("condT", [DC, T], BF16, kind="ExternalInput").ap()
    p1w = nc.dram_tensor("p1w", [DC, 2 * D], BF16, kind="ExternalInput").ap()
    p1b = nc.dram_tensor("p1b", [128, 16], F32, kind="ExternalInput").ap()
    qkvw = nc.dram_tensor("qkvw", [D, 3 * D], BF16, kind="ExternalInput").ap()
    wo = nc.dram_tensor("wo", [D, D], BF16, kind="ExternalInput").ap()
    p2w = nc.dram_tensor("p2w", [DC, 2 * D], BF16, kind="ExternalInput").ap()
    p2b = nc.dram_tensor("p2b", [128, 16], F32, kind="ExternalInput").ap()
    w1 = nc.dram_tensor("w1", [D, FF], BF16, kind="ExternalInput").ap()
    b1 = nc.dram_tensor("b1", [128, 32], F32, kind="ExternalInput").ap()
    w2 = nc.dram_tensor("w2", [FF, D], BF16, kind="ExternalInput").ap()
    b2 = nc.dram_tensor("b2", [128, 8], F32, kind="ExternalInput").ap()
    out_d = nc.dram_tensor("out", [D, T], F32, kind="ExternalOutput").ap()

    with tile.TileContext(nc) as tc:
        _emit(nc, tc, xT, condT, p1w, p1b, qkvw, wo, p2w, p2b, w1, b1, w2, b2, out_d)

    nc.compile()
    return nc


def _emit(nc, tc, xT, condT, p1w, p1b, qkvw, wo, p2w, p2b, w1, b1, w2, b2, out_d):
    # Pool lifetimes follow a two-sided stack discipline (LIFO per side):
    # left = phase-nested pools, right = phase-crossing carries.
    def pool(name, bufs=1, space="SBUF", side=None):
        return tc.alloc_tile_pool(name=name, bufs=bufs, space=space, side=side)

    # ---------------- persistent pools ----------------
    const = pool("const")
    work = pool("work", bufs=4)            # [128,T] temporaries
    small = pool("small", bufs=4)          # [1,T] stats
    dram = pool("dram", bufs=1, space="DRAM")

    # right-side carry: x lives to the end; the attention residual adds into
    # it in place, so it doubles as x1.
    x_pool = pool("x_pool", side="right")
    xT_sb = x_pool.tile([128, 8 * T], F32, name="xT_sb")

    # ---------------- warm-up collective ----------------
    # The first AllGather of a NEFF runs far below link rate (ring/descriptor
    # cold start). Fire a tiny one immediately; it has no input dependencies
    # (the data is junk) and overlaps the input DMAs.
    wu_in = dram.tile([128, 16], BF16, name="wu_in")
    wu_out = dram.tile([GROUP, 128, 16], BF16, name="wu_out")
    nc.gpsimd.collective_compute(
        "AllGather",
        ALU.bypass,
        replica_groups=[[0, 1, 2, 3], [4, 5, 6, 7]],
        ins=[wu_in[:]],
        outs=[wu_out[:]],
    )

    # ---------------- constants ----------------
    ones_col_bf = const.tile([128, 1], BF16, name="ones_col_bf")
    nc.vector.memset(ones_col_bf[:], 1.0)
    ones_row_f = const.tile([1, 128], F32, name="ones_row_f")
    nc.vector.memset(ones_row_f[:], 1.0)
    eps_t = const.tile([1, 1], F32, name="eps_t")
    nc.vector.memset(eps_t[:], EPS)
    ones_all = const.tile([128, 64], F32, name="ones_all")
    nc.vector.memset(ones_all[:], 1.0)

    p1b_sb = const.tile([128, 16], F32, name="p1b_sb")
    nc.sync.dma_start(p1b_sb[:], p1b)
    p2b_sb = const.tile([128, 16], F32, name="p2b_sb")
    nc.sync.dma_start(p2b_sb[:], p2b)
    b1_sb = const.tile([128, 32], F32, name="b1_sb")
    nc.sync.dma_start(b1_sb[:], b1)
    b2_sb = const.tile([128, 8], F32, name="b2_sb")
    nc.sync.dma_start(b2_sb[:], b2)

    # ---------------- input loads, priority order ----------------
    # cond gates silu -> gb1; x gates the adaln1 stats; p1w gates gb1;
    # qkvw is needed ~20us in (kv chunk matmuls); p2w only at ~45us.
    cond_pool = pool("cond_pool")
    cond_sb = cond_pool.tile([128, 4 * T], BF16, name="cond_sb")
    for a in range(4):
        nc.sync.dma_start(cond_sb[:, T * a:T * (a + 1)], condT[128 * a:128 * (a + 1), :])
    sc_sb = cond_pool.tile([128, 4 * T], BF16, name="sc_sb")

    for a in range(8):
        nc.sync.dma_start(xT_sb[:, T * a:T * (a + 1)], xT[128 * a:128 * (a + 1), :])

    proj_pool = pool("proj_pool")
    p1w_sb = proj_pool.tile([128, 4 * 2048], BF16, name="p1w_sb")
    for half in range(2):
        for a in range(4):
            nc.sync.dma_start(
                p1w_sb[:, 2048 * a + 1024 * half: 2048 * a + 1024 * (half + 1)],
                p1w[128 * a:128 * (a + 1), 1024 * half:1024 * (half + 1)])

    qkvw_pool = pool("qkvw_pool")
    qkvw_sb = qkvw_pool.tile([128, 8 * 3072], BF16, name="qkvw_sb")
    for a in range(8):
        nc.sync.dma_start(qkvw_sb[:, 3072 * a:3072 * (a + 1)], qkvw[128 * a:128 * (a + 1), :])

    p2w_sb = proj_pool.tile([128, 4 * 2048], BF16, name="p2w_sb")
    for a in range(4):
        nc.sync.dma_start(p2w_sb[:, 2048 * a:2048 * (a + 1)], p2w[128 * a:128 * (a + 1), :])

    # PSUM pools for the pre-phase: evictions (3) + stats (2) + broadcast (2)
    st_ps = pool("st_ps", bufs=2, space="PSUM")     # [1,T] stats accumulators
    bc_ps = pool("bc_ps", bufs=1, space="PSUM")     # [128, 2T] mu/rs broadcast
    mm_ps = pool("mm_ps", bufs=3, space="PSUM")     # matmul eviction banks

    # ---------------- helpers ----------------
    def adaln(src_f32, gb_sb, h_sb, tmp_pool, stp, bcp, prefix):
        """src_f32: [128, 8*T] f32 ([D, T] transposed); writes h_sb bf16.

        LayerNorm stats via PE ones-matmuls, then normalize+affine applied
        in [128, 2*T] j-pairs (gamma tiles 0..7 and beta tiles 8..15 of
        gb_sb are each contiguous, so pairs slice cleanly).
        """
        src_bf = tmp_pool.tile([128, 8 * T], BF16, name=f"{prefix}src_bf", tag="src_bf")
        sq = tmp_pool.tile([128, 8 * T], BF16, name=f"{prefix}sq", tag="sq")
        # quarter-granularity so the stats matmuls chase the producer
        # (x DMA chunks / wo evictions) instead of waiting for half the tile
        for hv in range(4):
            hs = slice(2 * T * hv, 2 * T * (hv + 1))
            nc.vector.tensor_copy(src_bf[:, hs], src_f32[:, hs])
            nc.vector.tensor_mul(sq[:, hs], src_bf[:, hs], src_bf[:, hs])

        sums = stp.tile([1, T], F32, name="sums", tag="st")
        for j in range(8):
            nc.tensor.matmul(sums[:], ones_col_bf[:], src_bf[:, j * T:(j + 1) * T],
                             start=(j == 0), stop=(j == 7))
        sumsq = stp.tile([1, T], F32, name="sumsq", tag="st")
        for j in range(8):
            nc.tensor.matmul(sumsq[:], ones_col_bf[:], sq[:, j * T:(j + 1) * T],
                             start=(j == 0), stop=(j == 7))

        mu = small.tile([1, T], F32, name="mu", tag="sm")
        nc.vector.tensor_scalar_mul(mu[:], sums[:], 1.0 / D)
        musq = small.tile([1, T], F32, name="musq", tag="sm")
        nc.vector.tensor_mul(musq[:], mu[:], mu[:])
        var = small.tile([1, T], F32, name="var", tag="sm")
        nc.vector.scalar_tensor_tensor(var[:], sumsq[:], 1.0 / D, musq[:],
                                       op0=ALU.mult, op1=ALU.subtract)
        lnv = small.tile([1, T], F32, name="lnv", tag="sm")
        nc.scalar.activation(lnv[:], var[:], AF.Ln, bias=eps_t[:], scale=1.0)
        rs = small.tile([1, T], F32, name="rs", tag="sm")
        nc.scalar.activation(rs[:], lnv[:], AF.Exp, scale=-0.5)

        # broadcast mu and rs to 128 partitions with one matmul each; the
        # x2 repeat the j-pair ops need comes from duplicated casts (cheaper
        # than extra PE broadcasts, and the psum tile shrinks to 2 banks)
        mrs_b = bcp.tile([128, 2 * T], F32, name="mrs_b", tag="bc")
        nc.tensor.matmul(mrs_b[:, 0:T], ones_row_f[:], mu[:], start=True, stop=True)
        nc.tensor.matmul(mrs_b[:, T:2 * T], ones_row_f[:], rs[:], start=True, stop=True)
        mrs_bs = tmp_pool.tile([128, 4 * T], BF16, name=f"{prefix}mrs_bs", tag="mrs")
        for r in range(2):
            nc.vector.tensor_copy(mrs_bs[:, r * T:(r + 1) * T], mrs_b[:, 0:T])
            nc.vector.tensor_copy(mrs_bs[:, (2 + r) * T:(3 + r) * T], mrs_b[:, T:2 * T])
        mu2 = mrs_bs[:, 0:2 * T]
        rs2 = mrs_bs[:, 2 * T:4 * T]

        for jp in range(0, 8, 2):
            sl = slice(jp * T, (jp + 2) * T)
            bsl = slice((8 + jp) * T, (10 + jp) * T)
            t1 = work.tile([128, 2 * T], BF16, name="t1", tag="wk2")
            nc.vector.tensor_sub(t1[:], src_bf[:, sl], mu2)
            t2 = work.tile([128, 2 * T], BF16, name="t2", tag="wk2")
            nc.vector.tensor_mul(t2[:], t1[:], rs2)
            t3 = work.tile([128, 2 * T], BF16, name="t3", tag="wk2")
            nc.vector.tensor_mul(t3[:], t2[:], gb_sb[:, sl])           # *(1+gamma)
            nc.vector.tensor_add(h_sb[:, sl], t3[:], gb_sb[:, bsl])

    def proj_gb(w_sb, b_sb, gb_sb):
        """gb^T = (silu(cond) @ W + b)^T : 16 M-tiles of [128, T]."""
        for m in range(16):
            ps = mm_ps.tile([128, T], F32, name="gbps", tag="mm")
            for k in range(4):
                nc.tensor.matmul(ps[:], w_sb[:, 2048 * k + 128 * m: 2048 * k + 128 * (m + 1)],
                                 sc_sb[:, k * T:(k + 1) * T],
                                 start=(k == 0), stop=(k == 3))
            nc.vector.tensor_scalar_add(gb_sb[:, m * T:(m + 1) * T], ps[:],
                                        b_sb[:, m:m + 1])

    # ---------------- AdaLN 1 ----------------
    for a in range(4):
        sl = slice(T * a, T * (a + 1))
        nc.scalar.activation(sc_sb[:, sl], cond_sb[:, sl], AF.Silu)

    h1_pool = pool("h1_pool")
    h1_sb = h1_pool.tile([128, 8 * T], BF16, name="h1_sb")

    gb1_pool = pool("gb1_pool")
    gb1_sb = gb1_pool.tile([128, 16 * T], BF16, name="gb1_sb")
    proj_gb(p1w_sb, p1b_sb, gb1_sb)

    aln1_tmp = pool("aln1_tmp")
    adaln(xT_sb, gb1_sb, h1_sb, aln1_tmp, st_ps, bc_ps, "a1")
    aln1_tmp.release()
    gb1_pool.release()

    # ---------------- qkv + chunked collective ----------------
    # 4 AllGather chunks, one per 4-head group: chunk c carries k^T feature
    # rows [256c:256c+256] and v columns [256c:256c+256], so attention on
    # head-pairs 2c,2c+1 can start while later chunks are still in flight.
    kv_pool = pool("kv_pool")
    kT_loc = kv_pool.tile([128, 8 * T], BF16, name="kT_loc")
    v_loc = kv_pool.tile([128, 4 * D], BF16, name="v_loc")

    NCH = 4
    kv_ins = [dram.tile([512, T], BF16, name=f"kv_in{c}") for c in range(NCH)]
    kv_outs = [dram.tile([GROUP, 512, T], BF16, name=f"kv_out{c}") for c in range(NCH)]

    for c in range(NCH):
        # k^T feature M-tiles for heads 4c..4c+3
        for ml in range(2):
            m = 8 + 2 * c + ml
            ps = mm_ps.tile([128, T], F32, name="kps", tag="mm")
            for k in range(8):
                nc.tensor.matmul(ps[:], qkvw_sb[:, 3072 * k + 128 * m: 3072 * k + 128 * (m + 1)],
                                 h1_sb[:, k * T:(k + 1) * T],
                                 start=(k == 0), stop=(k == 7))
            nc.vector.tensor_copy(kT_loc[:, (m - 8) * T:(m - 7) * T], ps[:])
        # v quarter c ([tokens, 256 features]), token M-tiles
        for mt in range(4):
            ps = mm_ps.tile([128, 256], F32, name="vps", tag="mm")
            for k in range(8):
                nc.tensor.matmul(
                    ps[:],
                    h1_sb[:, k * T + 128 * mt: k * T + 128 * (mt + 1)],
                    qkvw_sb[:, 3072 * k + 2048 + 256 * c: 3072 * k + 2048 + 256 * (c + 1)],
                    start=(k == 0), stop=(k == 7))
            nc.vector.tensor_copy(v_loc[:, 1024 * mt + 256 * c: 1024 * mt + 256 * (c + 1)], ps[:])
        # bounce writes + collective for this chunk
        for ml in range(2):
            nc.sync.dma_start(kv_ins[c][128 * ml:128 * (ml + 1), :],
                              kT_loc[:, (2 * c + ml) * T:(2 * c + ml + 1) * T])
        vdst = kv_ins[c][256:512, :].rearrange("r (two f) -> (r two) f", two=2)
        nc.sync.dma_start(vdst.rearrange("(m p) f -> p m f", m=4),
                          v_loc.rearrange("p (m f) -> p m f", m=4)[:, :, 256 * c:256 * (c + 1)])
        nc.gpsimd.collective_compute(
            "AllGather",
            ALU.bypass,
            replica_groups=[[0, 1, 2, 3], [4, 5, 6, 7]],
            ins=[kv_ins[c][:]],
            outs=[kv_outs[c][:]],
        )
    kv_pool.release()

    # right-side carries for the attention phase
    gb2_pool = pool("gb2_pool", side="right")
    gb2_sb = gb2_pool.tile([128, 16 * T], BF16, name="gb2_sb")
    oT_pool = pool("oT_pool", side="right")
    oT_sb = oT_pool.tile([128, 8 * T], BF16, name="oT_sb")
    q_pool = pool("q_pool", side="right")
    qT_sb = q_pool.tile([128, 8 * T], BF16, name="qT_sb")

    # q^T (feature M-tiles 0..7), overlaps with collective
    for m in range(8):
        ps = mm_ps.tile([128, T], F32, name="qps", tag="mm")
        for k in range(8):
            nc.tensor.matmul(ps[:], qkvw_sb[:, 3072 * k + 128 * m: 3072 * k + 128 * (m + 1)],
                             h1_sb[:, k * T:(k + 1) * T],
                             start=(k == 0), stop=(k == 7))
        nc.vector.tensor_copy(qT_sb[:, m * T:(m + 1) * T], ps[:])

    # gb2 projection, overlaps with collective
    proj_gb(p2w_sb, p2b_sb, gb2_sb)

    h1_pool.release()
    mm_ps.release()
    bc_ps.release()
    st_ps.release()
    qkvw_pool.release()
    proj_pool.release()
    cond_pool.release()

    # ---------------- attention ----------------
    att_pool = pool("att_pool")
    kT_full = att_pool.tile([128, 8 * S], BF16, name="kT_full")
    VW = DH + 1  # 65: per-head V columns + ones column (softmax denominator)
    vext = att_pool.tile([128, 16 * H * VW], BF16, name="vext")
    vext_v = vext.rearrange("p (c h m) -> p c h m", c=16, m=VW)
    nc.vector.memset(vext_v[:, :, :, DH:DH + 1], 1.0)

    def readback_chunk(c):
        for fl in range(2):
            f = 2 * c + fl
            for r in range(GROUP):
                nc.sync.dma_start(kT_full[:, 2048 * f + 512 * r: 2048 * f + 512 * (r + 1)],
                                  kv_outs[c][r, 128 * fl:128 * (fl + 1), :])
        for r in range(GROUP):
            vch = kv_outs[c][r, 256:512, :].rearrange("q (two f) -> (q two) f", two=2)
            for lc in range(4):
                c2 = 4 * r + lc
                src = vch[128 * lc:128 * (lc + 1), :].rearrange("t (h d) -> t h d", d=DH)
                # SWDGE queue: keeps vext readbacks off the sync DMA queues so
                # they don't serialize behind later chunks' waits
                nc.gpsimd.dma_start(vext_v[:, c2, 4 * c:4 * (c + 1), 0:DH], src)

    # emit all readbacks now, BEFORE any later weight loads: each chunk's
    # transfers wait on its AllGather semaphore, and weight bytes queued
    # ahead of them would delay the attention-critical data
    for c in range(NCH):
        readback_chunk(c)

    p_pool = pool("p_pool", bufs=4)
    norm_pool = pool("norm_pool", bufs=2)
    sc_ps = pool("sc_ps", bufs=3, space="PSUM")     # [128,1024] = 2 banks each
    o_ps_pool = pool("o_ps", bufs=2, space="PSUM")

    # Persistent den/rec ping-pong pairs: both heads' denominators live in
    # ONE tile, at partitions 64 (head 0) and 96 (head 1), so a single
    # fast-reciprocal + a single sel-matrix matmul + one [128,T] multiply
    # normalize a whole head pair (was: 2 matmuls + 2 casts + 2 muls).
    # den rows are primed to 1.0 once, so the junk rows stay finite through
    # the full-tile reciprocal and contribute sel=0 * finite = 0.
    den_t = [norm_pool.tile([128, T], F32, name=f"den{i}", tag=f"den{i}", bufs=1)
             for i in range(2)]
    rec_t = [norm_pool.tile([128, T], F32, name=f"rec{i}", tag=f"rec{i}", bufs=1)
             for i in range(2)]
    for i in range(2):
        nc.vector.memset(den_t[i][:], 1.0)
    # sel_b: contraction rows 64/96 select rec rows 64/96 into output halves
    sel_b = att_pool.tile([128, 128], F32, name="sel_b")
    nc.vector.memset(sel_b[:], 0.0)
    nc.vector.memset(sel_b[64:65, 0:64], 1.0)
    nc.vector.memset(sel_b[96:97, 64:128], 1.0)

    norm_pending = []

    def after_av(pv_hp, o_tiles):
        # Part A (DVE only): evict raw o^T + denominators (freeing o psum
        # quickly) and compute one fast-approx reciprocal for both heads; the
        # PE-side broadcast runs a pair later via flush_norm so the
        # reciprocal chain never stalls the in-order PE queue.
        den = den_t[pv_hp % 2]
        rec = rec_t[pv_hp % 2]
        for hh in range(2):
            nc.vector.tensor_copy(oT_sb[64 * hh:64 * (hh + 1), pv_hp * T:(pv_hp + 1) * T],
                                  o_tiles[hh][0:DH, :])
            nc.vector.tensor_copy(den[64 + 32 * hh:65 + 32 * hh, :],
                                  o_tiles[hh][DH:DH + 1, :])
        # full-tile op: the custom-DVE lowering needs a partition-0 start
        nc.vector.reciprocal_approx_fast(out=rec[:], in_=den[:])
        norm_pending.append((pv_hp, rec))

    def flush_norm():
        for (php, rec) in norm_pending:
            rbt = sc_ps.tile([128, 2 * T], F32, name="rbt", tag="s")
            nc.tensor.matmul(rbt[:, 0:T], sel_b[64:128, :], rec[64:128, :],
                             start=True, stop=True)
            rb_sb = norm_pool.tile([128, T], BF16, name="rb_sb", tag="rbs")
            nc.vector.tensor_copy(rb_sb[:], rbt[:, 0:T])
            osl = oT_sb[:, php * T:(php + 1) * T]
            nc.vector.tensor_mul(osl, osl, rb_sb[:])
        norm_pending.clear()

    prev = None
    for hp in range(8):
        p_tiles = [p_pool.tile([128, 16 * T], BF16, name=f"pt{hh}", tag="p") for hh in range(2)]
        q_h = [qT_sb[64 * hh:64 * (hh + 1), hp * T:(hp + 1) * T] for hh in range(2)]
        o_tiles = None
        if prev is not None:
            o_tiles = [o_ps_pool.tile([128, T], F32, name="o_ps", tag="o") for _ in range(2)]
        # 8 groups: scores for chunks (2m2, 2m2+1) of both heads, interleaved
        # with 4 AV matmuls of the previous pair so PE work overlaps ACT exp.
        for m2 in range(8):
            scts = [sc_ps.tile([128, 1024], F32, name="sct", tag="s") for _ in range(2)]
            for half in range(2):
                m = 2 * m2 + half
                for hh in range(2):
                    rows = slice(64 * hh, 64 * (hh + 1))
                    nc.tensor.matmul(
                        scts[hh][:, 512 * half:512 * (half + 1)],
                        kT_full[rows, 2048 * hp + 128 * m: 2048 * hp + 128 * (m + 1)],
                        q_h[hh],
                        start=True, stop=True)
            if prev is not None:
                pv_tiles, pv_hp = prev
                for hh in range(2):
                    h = 2 * pv_hp + hh
                    for half in range(2):
                        cc = 2 * m2 + half
                        nc.tensor.matmul(
                            o_tiles[hh][0:VW, :],
                            vext[:, VW * (16 * cc + h): VW * (16 * cc + h) + VW],
                            pv_tiles[hh][:, cc * T:(cc + 1) * T],
                            start=(cc == 0), stop=(cc == 15))
            for hh in range(2):
                nc.scalar.activation(p_tiles[hh][:, 2 * m2 * T:(2 * m2 + 2) * T],
                                     scts[hh][:], AF.Exp)
        flush_norm()
        if prev is not None:
            after_av(prev[1], o_tiles)
        prev = (p_tiles, hp)

    # tail: AV + normalize for the last pair
    pv_tiles, pv_hp = prev
    o_tiles = [o_ps_pool.tile([128, T], F32, name="o_ps", tag="o") for _ in range(2)]
    for cc in range(16):
        for hh in range(2):
            h = 2 * pv_hp + hh
            nc.tensor.matmul(
                o_tiles[hh][0:VW, :],
                vext[:, VW * (16 * cc + h): VW * (16 * cc + h) + VW],
                pv_tiles[hh][:, cc * T:(cc + 1) * T],
                start=(cc == 0), stop=(cc == 15))
    flush_norm()
    after_av(pv_hp, o_tiles)
    flush_norm()

    o_ps_pool.release()
    sc_ps.release()
    norm_pool.release()
    p_pool.release()
    att_pool.release()

    # ---------------- attn_out + residual (in place into xT_sb) ----------
    mm_ps2 = pool("mm_ps2", bufs=3, space="PSUM")
    st_ps2 = pool("st_ps2", bufs=2, space="PSUM")
    bc_ps2 = pool("bc_ps2", bufs=1, space="PSUM")

    wo_pool = pool("wo_pool")
    wo_sb = wo_pool.tile([128, 8 * D], BF16, name="wo_sb")
    for a in range(8):
        nc.sync.dma_start(wo_sb[:, 1024 * a:1024 * (a + 1)], wo[128 * a:128 * (a + 1), :])

    for m in range(8):
        ps = mm_ps2.tile([128, T], F32, name="aops", tag="mm")
        for k in range(8):
            nc.tensor.matmul(ps[:], wo_sb[:, 1024 * k + 128 * m: 1024 * k + 128 * (m + 1)],
                             oT_sb[:, k * T:(k + 1) * T],
                             start=(k == 0), stop=(k == 7))
        nc.vector.tensor_add(xT_sb[:, m * T:(m + 1) * T], ps[:], xT_sb[:, m * T:(m + 1) * T])
    wo_pool.release()

    q_pool.release()
    oT_pool.release()

    # ---------------- AdaLN 2 ----------------
    g_pool = pool("g_pool")
    g_sb = g_pool.tile([128, 32 * T], BF16, name="g_sb")

    h2_pool = pool("h2_pool")
    h2_sb = h2_pool.tile([128, 8 * T], BF16, name="h2_sb")

    w1_pool = pool("w1_pool")
    w1_sb = w1_pool.tile([128, 8 * FF], BF16, name="w1_sb")
    for a in range(8):
        nc.sync.dma_start(w1_sb[:, 4096 * a:4096 * (a + 1)], w1[128 * a:128 * (a + 1), :])

    aln2_tmp = pool("aln2_tmp")
    adaln(xT_sb, gb2_sb, h2_sb, aln2_tmp, st_ps2, bc_ps2, "a2")
    aln2_tmp.release()
    gb2_pool.release()
    bc_ps2.release()
    st_ps2.release()

    # ---------------- FFN ----------------
    for m in range(32):
        ps = mm_ps2.tile([128, T], F32, name="f1ps", tag="mm")
        for k in range(8):
            nc.tensor.matmul(ps[:], w1_sb[:, 4096 * k + 128 * m: 4096 * k + 128 * (m + 1)],
                             h2_sb[:, k * T:(k + 1) * T],
                             start=(k == 0), stop=(k == 7))
        nc.scalar.activation(g_sb[:, m * T:(m + 1) * T], ps[:], AF.Gelu,
                             bias=b1_sb[:, m:m + 1], scale=1.0)
    w1_pool.release()
    h2_pool.release()
    mm_ps2.release()

    # ffn2: k-outer, stream w2 k-tiles; two m-halves so the first half's
    # evictions overlap the second half's matmuls
    w2_pool = pool("w2_pool", bufs=8)
    ff2_ps = pool("ff2_ps", bufs=1, space="PSUM")
    out_pool0 = pool("out_pool0")
    out_sb = out_pool0.tile([128, 8 * T], F32, name="out_sb")
    for half in range(2):
        o2 = [ff2_ps.tile([128, T], F32, name=f"ff2_{m}", tag=f"ff2_{m}") for m in range(4)]
        for k in range(32):
            w2t = w2_pool.tile([128, 512], BF16, name="w2t", tag="w2t")
            nc.sync.dma_start(w2t[:], w2[128 * k: 128 * (k + 1), 512 * half:512 * (half + 1)])
            for m in range(4):
                nc.tensor.matmul(o2[m][:], w2t[:, 128 * m: 128 * (m + 1)],
                                 g_sb[:, k * T:(k + 1) * T],
                                 start=(k == 0), stop=(k == 31))
        for m in range(4):
            gm = 4 * half + m
            nc.vector.scalar_tensor_tensor(out_sb[:, gm * T:(gm + 1) * T], o2[m][:],
                                           b2_sb[:, gm:gm + 1], xT_sb[:, gm * T:(gm + 1) * T],
                                           op0=ALU.add, op1=ALU.add)
        for a in range(4 * half, 4 * half + 4):
            nc.sync.dma_start(out_d[128 * a:128 * (a + 1), :], out_sb[:, T * a:T * (a + 1)])

    out_pool0.release()
    ff2_ps.release()
    w2_pool.release()
    g_pool.release()
    x_pool.release()
    small.release()
    work.release()
    const.release()
    dram.release()


def _bf16(a):
    return np.ascontiguousarray(a).astype(ml_dtypes.bfloat16)


def _prep_maps(x, cond, p1_w, p1_b, qkv_w, attn_out_w, p2_w, p2_b,
               ffn_w1, ffn_b1, ffn_w2, ffn_b2):
    x = np.asarray(x, np.float32)
    cond = np.asarray(cond, np.float32)
    qkv_mod = np.asarray(qkv_w, np.float32).copy()
    qkv_mod[:, :D] *= DH ** -0.5                      # fold 1/sqrt(d) into q
    p1b_mod = np.asarray(p1_b, np.float32).copy()
    p1b_mod[:D] += 1.0                                # fold AdaLN "+1" into gamma bias
    p2b_mod = np.asarray(p2_b, np.float32).copy()
    p2b_mod[:D] += 1.0

    shared = {
        "p1w": _bf16(p1_w),
        "p1b": np.ascontiguousarray(p1b_mod.reshape(16, 128).T, np.float32),
        "qkvw": _bf16(qkv_mod),
        "wo": _bf16(attn_out_w),
        "p2w": _bf16(p2_w),
        "p2b": np.ascontiguousarray(p2b_mod.reshape(16, 128).T, np.float32),
        "w1": _bf16(ffn_w1),
        "b1": np.ascontiguousarray(np.asarray(ffn_b1, np.float32).reshape(32, 128).T,
                                   np.float32),
        "w2": _bf16(ffn_w2),
        "b2": np.ascontiguousarray(np.asarray(ffn_b2, np.float32).reshape(8, 128).T,
                                   np.float32),
    }
    in_maps = []
    for core in range(NCORES):
        b, r = core // GROUP, core % GROUP
        sl = slice(T * r, T * (r + 1))
        m = dict(shared)
        m["xT"] = np.ascontiguousarray(x[b, sl, :].T, np.float32)
        m["condT"] = _bf16(cond[b, sl, :].T)
        in_maps.append(m)
    return in_maps


def _get_nc():
    if "nc" not in _CACHE:
        _CACHE["nc"] = _build()
    return _CACHE["nc"]


def _install_ntff_hook():
    """This image's antenv lacks axon_hooks; recreate it (see trn_boot.py)."""
    import sys, types, ctypes, contextlib

    if "antenv.axon_hooks" in sys.modules:
        return
    mod = types.ModuleType("antenv.axon_hooks")
    state = {"hook": None}
    mod.set_axon_ntff_profile_hook = lambda h: state.__setitem__("hook", h)
    mod.get_axon_ntff_profile_hook = lambda: state["hook"]
    sys.modules["antenv.axon_hooks"] = mod
    try:
        import antenv
        antenv.axon_hooks = mod
    except ImportError:
        pass

    so_path = "/opt/axon/libaxon_pjrt.so"
    if not os.path.exists(so_path):
        return
    lib = ctypes.CDLL(so_path)
    if not hasattr(lib, "axon_start_nrt_profile"):
        return
    lib.axon_start_nrt_profile.argtypes = [ctypes.POINTER(ctypes.c_int64), ctypes.c_size_t]
    lib.axon_start_nrt_profile.restype = ctypes.c_int64
    lib.axon_stop_nrt_profile.argtypes = [ctypes.c_char_p]
    lib.axon_stop_nrt_profile.restype = ctypes.c_int64

    @contextlib.contextmanager
    def _hook(output_dir, device_ids):
        import jax
        jax.devices()
        if device_ids:
            ids = (ctypes.c_int64 * len(device_ids))(*device_ids)
            rc = lib.axon_start_nrt_profile(ids, len(device_ids))
        else:
            rc = lib.axon_start_nrt_profile(None, 0)
        if rc != 0:
            raise RuntimeError(f"axon_start_nrt_profile rc={rc}")
        try:
            yield
        finally:
            n = lib.axon_stop_nrt_profile(str(output_dir).encode())
            print(f"ntff profile: {n} file(s) -> {output_dir}")

    mod.set_axon_ntff_profile_hook(_hook)


def run(in_maps, trace=False, **kw):
    if trace:
        _install_ntff_hook()
    nc = _get_nc()
    return run_bass_kernel_spmd(nc, in_maps, core_ids=list(range(NCORES)),
                                trace=trace, **kw)


def kernel(**inputs):
    in_maps = _prep_maps(**inputs)
    res = run(in_maps).results
    out = np.empty((B, S, D), np.float32)
    for core in range(NCORES):
        b, r = core // GROUP, core % GROUP
        out[b, T * r: T * (r + 1), :] = res[core]["out"].T
    return out
